# revision 1
# baseline (speedup 1.0000x reference)
"""GRU greedy decoder on 8 trn2 cores.

Vocab-sharded: each core owns 4000 vocab entries of the out-projection; per
step it computes its local (B=32, 4000) logits, finds the local argmax, all
cores exchange (max-value, global-index) candidates, everyone computes the
global argmax, gathers the fused embedding row G'[tok] = emb[tok]@W_ih.T+b_ih,
and advances the replicated GRU state. Logits are staged to SBUF and DMA'd to
each core's output stripe.

Layout:
  PSUM logits tile [128, 1024]: partition 32j + m (j = psum col group, m =
  batch), free = pos in [0, 1024); vocab v = core*4000 + j*1000 + pos for
  pos < 1000; pos in [1000, 1024) are pad slots with bias -1e30.
  Matmuls: col-group tiling only (row groups != 0 crash at runtime on this
  stack): per j, two N=512 matmuls; lhsT = hT_aug [17, 32] at partitions 0:17.
"""

import numpy as np
import concourse.bass as bass
import concourse.bacc as bacc
import concourse.mybir as mybir
from concourse import tile
from concourse.bass import AP, IndirectOffsetOnAxis
from concourse.tile_rust import add_dep_helper

FP = mybir.dt.float32
B, H, E, V, T = 32, 16, 16, 32000, 100
NCORES = 8
VLOC = V // NCORES           # 4000
F = VLOC // 4                # 1000 valid entries per partition
FPAD = 1024                  # padded free size (PSUM bank alignment)
K = H + 1                    # 17: h dims + ones row (bias)
G3 = 3 * H                   # 48


def build_nc(n_steps=T, exchange="allgather", psum_max=True, reps=1,
             out_steps=None, mm_dtype=FP, xdma_pool=False):
    # out_steps: number of distinct output step-slots (timing runs use a small
    # value to shrink D2H transfer; DMA count/size per step is unchanged).
    # mm_dtype: dtype the big vocab matmul operands are bitcast to (FP or
    # float32r for the 1-cycle/row PE mode).
    if out_steps is None:
        out_steps = n_steps
    nc = bacc.Bacc(None, target_bir_lowering=False)

    # ---- I/O ----
    wt_in = nc.dram_tensor("wt", [K, 4 * FPAD], FP, kind="ExternalInput")
    whh_in = nc.dram_tensor("whh", [K, G3], FP, kind="ExternalInput")
    h0t_in = nc.dram_tensor("h0t", [K, B], FP, kind="ExternalInput")
    h0a_in = nc.dram_tensor("h0a", [B, K], FP, kind="ExternalInput")
    off2_in = nc.dram_tensor("off2", [128, 1], FP, kind="ExternalInput")
    ident_in = nc.dram_tensor("ident", [128, 128], FP, kind="ExternalInput")
    gp_in = nc.dram_tensor("gp", [V, G3], FP, kind="ExternalInput")
    out_dram = nc.dram_tensor("out", [B, out_steps, VLOC], FP, kind="ExternalOutput")

    with tile.TileContext(nc) as tc:
        with (
            tc.tile_pool(name="const", bufs=1) as cpool,
            tc.tile_pool(name="state", bufs=1) as spool,
            tc.tile_pool(name="work", bufs=4) as wpool,
            tc.tile_pool(name="stage", bufs=3) as stpool,
            tc.tile_pool(name="psumL", bufs=2, space="PSUM") as plpool,
            tc.tile_pool(name="psumG", bufs=1, space="PSUM") as pgpool,
            tc.tile_pool(name="psumT", bufs=1, space="PSUM") as ptpool,
            tc.tile_pool(name="dram", bufs=4, space="DRAM") as dpool,
        ):
            # ---- constants to SBUF ----
            wt_sb = cpool.tile([K, 4 * FPAD], FP, tag="wt")
            nc.gpsimd.dma_start(out=wt_sb[:, :], in_=wt_in[:, :])
            whh_sb = cpool.tile([K, G3], FP, tag="whh")
            nc.gpsimd.dma_start(out=whh_sb[:, :], in_=whh_in[:, :])
            off2_sb = cpool.tile([128, 1], FP, tag="off2")
            nc.gpsimd.dma_start(out=off2_sb[:, :], in_=off2_in[:, :])
            ident_sb = cpool.tile([128, 128], FP, tag="ident")
            nc.gpsimd.dma_start(out=ident_sb[:, :], in_=ident_in[:, :])

            # ---- state ----
            hT_sb = spool.tile([K, B], FP, tag="hT")     # hT_aug [17, 32]
            nc.gpsimd.dma_start(out=hT_sb[:, :], in_=h0t_in[:, :])
            if mm_dtype != FP:
                # one-time rounding of scan weights to f32r; per-step h round
                wt_r = cpool.tile([K, 4 * FPAD], mm_dtype, tag="wtr")
                nc.vector.tensor_copy(out=wt_r[:, :], in_=wt_sb[:, :])
            ha_sb = spool.tile([B, K], FP, tag="ha")     # h_aug [32, 17] col 16 = ones
            nc.gpsimd.dma_start(out=ha_sb[:, :], in_=h0a_in[:, :])
            vs32 = spool.tile([B, B], FP, tag="vs32")    # row 0 = selected idx
            nc.vector.memset(vs32[:, :], 0.0)

            RING = 4
            patches = []   # (BassInstruction, sem, threshold) applied post-schedule
            if exchange == "remote":
                rsem = nc.alloc_semaphore("rsem")
                lsem = nc.alloc_semaphore("lsem")
                rcvb = spool.tile([128, RING * 16], FP, tag="rcvb")
                pid = nc.gpsimd.partition_id()
                poff = pid * 2

            for rep in range(reps):
              for t in range(n_steps):
                tg = rep * n_steps + t
                # ---------- logits matmuls (col-group tiling) ----------
                if mm_dtype != FP:
                    hT_r = wpool.tile([K, B], mm_dtype, tag="hTr")
                    nc.vector.tensor_copy(out=hT_r[:, :], in_=hT_sb[:, :])
                    scan_lhs, scan_rhs = hT_r, wt_r
                else:
                    scan_lhs, scan_rhs = hT_sb, wt_sb
                lps = plpool.tile([128, FPAD], FP, tag="L")
                for j in range(4):
                    for hh in range(2):
                        nc.tensor.matmul(
                            out=lps[32 * j:32 * j + B, hh * 512:(hh + 1) * 512],
                            lhsT=scan_lhs[:, :],
                            rhs=scan_rhs[:, j * FPAD + hh * 512:j * FPAD + (hh + 1) * 512],
                            start=True, stop=True,
                            tile_position=(0, 32 * j),
                        )
                # gh = h @ W_hh.T + b_hh  -> [32, 48]
                ghp = pgpool.tile([B, G3], FP, tag="gh")
                nc.tensor.matmul(
                    out=ghp[:, :], lhsT=hT_sb[:, :], rhs=whh_sb[:, :],
                    start=True, stop=True, tile_position=(0, 0),
                )

                # ---------- stage to SBUF for output (4-step slabs) ----------
                if t % 4 == 0:
                    slab = stpool.tile([128, 4 * FPAD], FP, tag="stg4")
                stg = slab[:, (t % 4) * FPAD:(t % 4 + 1) * FPAD]
                nc.scalar.copy(out=stg, in_=lps[:, :])

                # ---------- local argmax ----------
                mx8 = wpool.tile([128, 8], FP, tag="mx8")
                mi8 = wpool.tile([128, 8], mybir.dt.uint32, tag="mi8")
                src = lps
                nc.vector.max(out=mx8[:, :], in_=src[:, :])
                nc.vector.max_index(out=mi8[:, :], in_max=mx8[:, :], in_values=src[:, :])

                # payload CW [128, 2] = (val, global idx as fp32)
                cw = wpool.tile([128, 2], FP, tag="cw")
                if exchange == "remote" and tg >= 3:
                    wn = nc.vector.nop(nofuse=True, hint="lsem_wait")
                    patches.append((wn, lsem, 16 * (tg - 2)))
                cwi1 = nc.vector.tensor_copy(out=cw[:, 0:1], in_=mx8[:, 0:1])
                cwi2 = nc.vector.tensor_scalar_add(cw[:, 1:2], mi8[:, 0:1], off2_sb[:, 0:1])
                if exchange == "remote" and tg >= 3:
                    add_dep_helper(wn.ins, cwi1.ins, sync=False, reason="lsem order")
                    add_dep_helper(wn.ins, cwi2.ins, sync=False, reason="lsem order")

                # ---------- exchange ----------
                if exchange.startswith("allgather"):
                    xeng = nc.gpsimd if xdma_pool else nc.scalar
                    cc_in = dpool.tile([128, 2], FP, tag="ccin")
                    cc_out = dpool.tile([128 * NCORES, 2], FP, tag="ccout")
                    xeng.dma_start(out=cc_in[:, :], in_=cw[:, :])
                    nc.gpsimd.collective_compute(
                        "AllGather",
                        mybir.AluOpType.bypass,
                        ins=[cc_in[:, :].opt()],
                        outs=[cc_out[:, :].opt()],
                        replica_groups=[list(range(NCORES))],
                    )
                    if exchange == "allgather2":
                        cc_out2 = dpool.tile([128 * NCORES, 2], FP, tag="ccout2")
                        nc.gpsimd.collective_compute(
                            "AllGather",
                            mybir.AluOpType.bypass,
                            ins=[cc_in[:, :].opt()],
                            outs=[cc_out2[:, :].opt()],
                            replica_groups=[list(range(NCORES))],
                        )
                    rcv = wpool.tile([128, NCORES * 2], FP, tag="rcv")
                    # iterate (p, c, x): strides in elements: p->2, c->256, x->1
                    xeng.dma_start(
                        out=rcv[:, :],
                        in_=AP(cc_out[:, :].tensor, 0,
                               [[2, 128], [128 * 2, NCORES], [1, 2]]),
                    )
                elif exchange == "remote":
                    slot = t % RING
                    out_ap = AP(rcvb[:, :].tensor, slot * 16 + poff,
                                [[RING * 16, 128], [1, 2]])
                    nc.gpsimd.remote_dma_broadcast(
                        out_ap=out_ap, in_ap=cw[:, :],
                        remote_sem=rsem, local_sem=lsem,
                        rdests=[(0, k) for k in range(NCORES)],
                    )
                    nc.gpsimd.trigger_dma(count=None)
                    rcv = rcvb[:, slot * 16:(slot + 1) * 16]
                elif exchange == "none":
                    rcv = None
                else:
                    raise ValueError(exchange)

                cw2 = wpool.tile([128, 2], FP, tag="cw2")
                if rcv is not None and exchange in ("allgather", "remote"):
                    # core-combine: best over 8 cores per (j, m) partition
                    rm8 = wpool.tile([128, 8], FP, tag="rm8")
                    if exchange == "remote":
                        rr = rcv.rearrange("p (c x) -> p c x", x=2)
                    else:
                        rr = rcv[:, :].rearrange("p (c x) -> p c x", x=2)
                    vals = rr[:, :, 0]
                    idxs = rr[:, :, 1]
                    if exchange == "remote":
                        wr = nc.vector.nop(nofuse=True, hint="rsem_wait")
                        patches.append((wr, rsem, 16 * (tg + 1)))
                    mxi = nc.vector.max(out=rm8[:, :], in_=vals)
                    if exchange == "remote":
                        add_dep_helper(wr.ins, mxi.ins, sync=False, reason="rsem order")
                    mskc = wpool.tile([128, 8], FP, tag="mskc")
                    nc.vector.tensor_scalar(
                        out=mskc[:, :], in0=vals, scalar1=rm8[:, 0:1], scalar2=None,
                        op0=mybir.AluOpType.is_equal,
                    )
                    tmpc = wpool.tile([128, 8], FP, tag="tmpc")
                    nc.vector.tensor_tensor(out=tmpc[:, :], in0=mskc[:, :], in1=idxs,
                                            op=mybir.AluOpType.mult)
                    gidxc = wpool.tile([128, 1], FP, tag="gidxc")
                    nc.vector.tensor_reduce(
                        out=gidxc[:, :], in_=tmpc[:, :], axis=mybir.AxisListType.X,
                        op=mybir.AluOpType.add,
                    )
                    cwv, cwi = rm8[:, 0:1], gidxc[:, :]
                else:
                    if rcv is not None:
                        nc.vector.tensor_copy(out=cw2[:, :], in_=rcv[:, 0:2])
                    else:
                        nc.vector.tensor_copy(out=cw2[:, :], in_=cw[:, :])
                    cwv, cwi = cw2[:, 0:1], cw2[:, 1:2]

                # ---------- j-combine ----------
                tj = ptpool.tile([1, 256], FP, tag="tj")
                tjv = tj[:, 0:128]
                tji = tj[:, 128:256]
                nc.tensor.transpose(out=tjv, in_=cwv, identity=ident_sb[:, :])
                nc.tensor.transpose(out=tji, in_=cwi, identity=ident_sb[:, :])
                gmj = wpool.tile([1, B], FP, tag="gmj")
                vrow = tjv.rearrange("p (j m) -> p m j", j=4)
                irow = tji.rearrange("p (j m) -> p m j", j=4)
                nc.vector.tensor_reduce(
                    out=gmj[:, :], in_=vrow, axis=mybir.AxisListType.X,
                    op=mybir.AluOpType.max,
                )
                msk2 = wpool.tile([1, 128], FP, tag="msk2")
                m2v = msk2[:, :].rearrange("p (j m) -> p m j", j=4)
                nc.vector.tensor_tensor(
                    out=m2v, in0=vrow,
                    in1=gmj[:, :].unsqueeze(2).to_broadcast([1, B, 4]),
                    op=mybir.AluOpType.is_equal,
                )
                tmp2 = wpool.tile([1, 128], FP, tag="tmp2")
                t2v = tmp2[:, :].rearrange("p (j m) -> p m j", j=4)
                nc.vector.tensor_tensor(out=t2v, in0=m2v, in1=irow,
                                        op=mybir.AluOpType.mult)
                nc.vector.tensor_reduce(
                    out=vs32[0:1, 0:B], in_=t2v, axis=mybir.AxisListType.X,
                    op=mybir.AluOpType.add,
                )
                # transpose row -> column, cast to int
                vs32t = wpool.tile([B, B], FP, tag="vs32t")
                nc.vector.transpose(out=vs32t[:, :], in_=vs32[:, :])
                idxi = wpool.tile([B, 1], mybir.dt.int32, tag="idxi")
                nc.vector.tensor_copy(out=idxi[:, :], in_=vs32t[:, 0:1])

                # ---------- gather G'[tok] ----------
                xg = wpool.tile([B, G3], FP, tag="xg")
                nc.gpsimd.indirect_dma_start(
                    out=xg[:, :], out_offset=None,
                    in_=gp_in[:, :],
                    in_offset=IndirectOffsetOnAxis(ap=idxi[:, 0:1], axis=0),
                )

                # ---------- GRU ----------
                rzp = wpool.tile([B, 2 * H], FP, tag="rzp")
                nc.vector.tensor_add(out=rzp[:, :], in0=xg[:, 0:2 * H], in1=ghp[:, 0:2 * H])
                rz = wpool.tile([B, 2 * H], FP, tag="rz")
                nc.scalar.activation(out=rz[:, :], in_=rzp[:, :],
                                     func=mybir.ActivationFunctionType.Sigmoid)
                rh = wpool.tile([B, H], FP, tag="rh")
                nc.vector.tensor_mul(out=rh[:, :], in0=rz[:, 0:H], in1=ghp[:, 2 * H:G3])
                npre = wpool.tile([B, H], FP, tag="npre")
                nc.vector.tensor_add(out=npre[:, :], in0=xg[:, 2 * H:G3], in1=rh[:, :])
                nn_ = wpool.tile([B, H], FP, tag="nn")
                nc.scalar.activation(out=nn_[:, :], in_=npre[:, :],
                                     func=mybir.ActivationFunctionType.Tanh)
                dd = wpool.tile([B, H], FP, tag="dd")
                nc.vector.tensor_sub(out=dd[:, :], in0=ha_sb[:, 0:H], in1=nn_[:, :])
                zd = wpool.tile([B, H], FP, tag="zd")
                nc.vector.tensor_mul(out=zd[:, :], in0=rz[:, H:2 * H], in1=dd[:, :])
                nc.vector.tensor_add(out=ha_sb[:, 0:H], in0=nn_[:, :], in1=zd[:, :])

                # hT update: transpose ha [32, 17] -> [17, 32]
                htt = ptpool.tile([K, B], FP, tag="htt")
                nc.tensor.transpose(out=htt[:, :], in_=ha_sb[:, :],
                                    identity=ident_sb[0:B, 0:B])
                nc.vector.tensor_copy(out=hT_sb[:, :], in_=htt[:, :])

                # ---------- output DMA ----------
                if t % 4 == 3 or t == n_steps - 1:
                    nt = t % 4 + 1
                    t0_ = t - nt + 1
                    for tt in range(nt):
                        dst = AP(out_dram, ((t0_ + tt) % out_steps) * VLOC,
                                 [[1000, 4], [out_steps * VLOC, B], [1, F]])
                        nc.scalar.dma_start(
                            out=dst, in_=slab[:, tt * FPAD:tt * FPAD + F])

    for bi, sem, thr in patches:
        bi.wait_op(sem, thr, "sem-ge")
    nc.finalize()
    return nc


def build_nc_v2(n_steps=T, reps=1, out_steps=None, mm_dtype=FP, debug=False,
                exchange="allgather", bc=True):
    """Lean critical path:
    - scan matmuls optionally float32r (mm_dtype)
    - sender-side j-fold to batch-major candidates; split val/idx remote
      broadcasts so max_index hides under the val exchange
    - receiver combine: 4 small DVE ops
    - transposed GRU state hT [17, 32] (partition 16 = ones), gates in one
      PSUM bank free-sliced at partitions 0:16; embedding rows transposed
      into PSUM via accumulating PE transposes; no final state transpose
    """
    if out_steps is None:
        out_steps = n_steps
    RING = 4
    nc = bacc.Bacc(None, target_bir_lowering=False)

    wt_in = nc.dram_tensor("wt", [K, 4 * FPAD], FP, kind="ExternalInput")
    whh_in = nc.dram_tensor("whh", [K, G3], FP, kind="ExternalInput")
    h0t_in = nc.dram_tensor("h0t", [K, B], FP, kind="ExternalInput")
    off2_in = nc.dram_tensor("off2", [128, 1], FP, kind="ExternalInput")
    ident_in = nc.dram_tensor("ident", [128, 128], FP, kind="ExternalInput")
    gp_in = nc.dram_tensor("gp", [V, G3], FP, kind="ExternalInput")
    out_dram = nc.dram_tensor("out", [B, out_steps, VLOC], FP, kind="ExternalOutput")
    if debug:
        # per step: bm row0 (64), xin cols 0+32 stacked (2*128), rcvV slot (8*32=256 as 128x2?),
        # dump flat: [n_steps, 64 + 256 + 256 + 256 + 32 + 32]
        dbg_dram = nc.dram_tensor("dbg", [n_steps, 64 + 128 * 2 + 256 + 256 + 32 + 32
                                           + 32 * 48 + 17 * 32 + 16 * 224],
                                  FP, kind="ExternalOutput")

    patches = []
    with tile.TileContext(nc) as tc:
        with (
            tc.tile_pool(name="const", bufs=1) as cpool,
            tc.tile_pool(name="state", bufs=1) as spool,
            tc.tile_pool(name="work", bufs=4) as wpool,
            tc.tile_pool(name="stage", bufs=3) as stpool,
            tc.tile_pool(name="psumL", bufs=2, space="PSUM") as plpool,
            tc.tile_pool(name="psumG", bufs=1, space="PSUM") as pgpool,
            tc.tile_pool(name="dram", bufs=4, space="DRAM") as dpool,
        ):
            # constants
            wt_sb = cpool.tile([K, 4 * FPAD], FP, tag="wt")
            nc.gpsimd.dma_start(out=wt_sb[:, :], in_=wt_in[:, :])
            whh_sb = cpool.tile([K, G3], FP, tag="whh")
            nc.gpsimd.dma_start(out=whh_sb[:, :], in_=whh_in[:, :])
            off2_sb = cpool.tile([128, 1], FP, tag="off2")
            nc.gpsimd.dma_start(out=off2_sb[:, :], in_=off2_in[:, :])
            ident_sb = cpool.tile([128, 128], FP, tag="ident")
            nc.gpsimd.dma_start(out=ident_sb[:, :], in_=ident_in[:, :])

            # state: hT_aug [17, 32], partition 16 stays all-ones
            hT_sb = spool.tile([K, B], FP, tag="hT")
            nc.gpsimd.dma_start(out=hT_sb[:, :], in_=h0t_in[:, :])

            # sender-side staging
            bm = spool.tile([B, 64], FP, tag="bm")
            nc.vector.memset(bm[:, :], 0.0)
            xin = spool.tile([128, 64], FP, tag="xin")
            nc.vector.memset(xin[:, :], 0.0)

            for rep in range(reps):
              for t in range(n_steps):
                tg = rep * n_steps + t
                # ---------- scan matmuls ----------
                lps = plpool.tile([128, FPAD], FP, tag="L")
                for j in range(4):
                    for hh in range(2):
                        nc.tensor.matmul(
                            out=lps[32 * j:32 * j + B, hh * 512:(hh + 1) * 512],
                            lhsT=hT_sb[:, :].bitcast(mm_dtype),
                            rhs=wt_sb[:, j * FPAD + hh * 512:j * FPAD + (hh + 1) * 512].bitcast(mm_dtype),
                            start=True, stop=True,
                            tile_position=(0, 32 * j),
                        )
                # gh gates (exact fp32): pt free-sliced [16, 128]:
                #   r: 0:32, z: 32:64, n(gh): 64:96, n(gi): 96:128
                # pt slots (512B-aligned matmul outs):
                # 0 gh_r | 128 gh_z | 256 gh_n | 384 gi_n | 512 gi_r | 640 gi_z
                pt = pgpool.tile([H, 768], FP, tag="pt")
                for g in range(3):
                    nc.tensor.matmul(
                        out=pt[:, 128 * g:128 * g + 32],
                        lhsT=whh_sb[:, 16 * g:16 * g + 16],
                        rhs=hT_sb[:, :],
                        start=True, stop=True, tile_position=(0, 0),
                    )

                # ---------- stage to SBUF for output ----------
                if t % 4 == 0:
                    slab = stpool.tile([128, 4 * FPAD], FP, tag="stg4")
                stg = slab[:, (t % 4) * FPAD:(t % 4 + 1) * FPAD]
                nc.scalar.copy(out=stg, in_=lps[:, :])
                # gh r,z to SBUF (off critical path; avoids 2-PSUM-operand op)
                ghsb = wpool.tile([H, 64], FP, tag="ghsb")
                nc.scalar.copy(out=ghsb[:, 0:32], in_=pt[:, 0:32])
                nc.scalar.copy(out=ghsb[:, 32:64], in_=pt[:, 128:160])

                # ---------- local argmax ----------
                mx8 = wpool.tile([128, 8], FP, tag="mx8")
                nc.vector.max(out=mx8[:, :], in_=lps[:, :])
                mi8 = wpool.tile([128, 8], mybir.dt.uint32, tag="mi8")
                nc.vector.max_index(out=mi8[:, :], in_max=mx8[:, :], in_values=lps[:, :])

                # ---------- sender j-fold (batch-major) ----------
                # tj [1, 256] psum: cols 0:128 = vals(j,m), 128:256 = gidx(j,m)
                tj = pgpool.tile([1, 256], FP, tag="tj")
                tjv = tj[:, 0:128]
                tji = tj[:, 128:256]
                nc.tensor.transpose(out=tjv, in_=mx8[:, 0:1], identity=ident_sb[:, :])
                cwi = wpool.tile([128, 1], FP, tag="cwi")
                nc.vector.tensor_scalar_add(cwi[:, :], mi8[:, 0:1], off2_sb[:, 0:1])
                nc.tensor.transpose(out=tji, in_=cwi[:, :], identity=ident_sb[:, :])

                # bm [32, 64]: row0 block0 = per-batch max val, row0 block1 = idx
                vrow = tjv.rearrange("p (j m) -> p m j", j=4)
                nc.vector.tensor_reduce(
                    out=bm[0:1, 0:B], in_=vrow, axis=mybir.AxisListType.X,
                    op=mybir.AluOpType.max,
                )
                msk = wpool.tile([1, 128], FP, tag="msk")
                mskv = msk[:, :].rearrange("p (j m) -> p m j", j=4)
                nc.vector.tensor_tensor(
                    out=mskv, in0=vrow,
                    in1=bm[0:1, 0:B].unsqueeze(2).to_broadcast([1, B, 4]),
                    op=mybir.AluOpType.is_equal,
                )
                tmpj = wpool.tile([1, 128], FP, tag="tmpj")
                tmpjv = tmpj[:, :].rearrange("p (j m) -> p m j", j=4)
                irow = tji.rearrange("p (j m) -> p m j", j=4)
                nc.vector.tensor_tensor(out=tmpjv, in0=mskv, in1=irow,
                                        op=mybir.AluOpType.mult)
                nc.vector.tensor_reduce(
                    out=bm[0:1, 32:32 + B], in_=tmpjv, axis=mybir.AxisListType.X,
                    op=mybir.AluOpType.add,
                )

                nc.vector.transpose(out=xin[0:B, 0:32], in_=bm[:, 0:32])
                nc.vector.transpose(out=xin[0:B, 32:64], in_=bm[:, 32:64])

                # ---------- exchange: AllGather of [128, 2] batch-major ----------
                # (rows 32:128 are padding; same collective shape as the
                # proven v1 path)
                rcvb = wpool.tile([B, 2 * NCORES], FP, tag="rcvb")
                if exchange == "allgather":
                    cc_in = dpool.tile([128, 2], FP, tag="ccin")
                    cc_out = dpool.tile([128 * NCORES, 2], FP, tag="ccout")
                    nc.scalar.dma_start(
                        out=cc_in[:, :],
                        in_=AP(xin[:, :].tensor, 0, [[64, 128], [32, 2]]))
                    nc.gpsimd.collective_compute(
                        "AllGather",
                        mybir.AluOpType.bypass,
                        ins=[cc_in[:, :].opt()],
                        outs=[cc_out[:, :].opt()],
                        replica_groups=[list(range(NCORES))],
                    )
                    nc.scalar.dma_start(
                        out=rcvb[:, :],
                        in_=AP(cc_out[:, :].tensor, 0,
                               [[2, B], [128 * 2, NCORES], [1, 2]]),
                    )
                else:
                    # timing-only: fake exchange, replicate own candidates
                    nc.vector.tensor_copy(
                        out=rcvb[:, :].rearrange("p (c x) -> p c x", x=2),
                        in_=AP(xin[:, :].tensor, 0,
                               [[64, B], [32, 2]]).unsqueeze(1).to_broadcast(
                                   [B, NCORES, 2]))

                # ---------- receiver combine ----------
                rr = rcvb[:, :].rearrange("p (c x) -> p c x", x=2)
                rv = wpool.tile([B, 1], FP, tag="rv")
                nc.vector.tensor_reduce(
                    out=rv[:, :], in_=rr[:, :, 0],
                    axis=mybir.AxisListType.X, op=mybir.AluOpType.max,
                )
                mskc = wpool.tile([B, NCORES], FP, tag="mskc")
                nc.vector.tensor_scalar(
                    out=mskc[:, :], in0=rr[:, :, 0],
                    scalar1=rv[:, 0:1], scalar2=None,
                    op0=mybir.AluOpType.is_equal,
                )
                junk = wpool.tile([B, NCORES], FP, tag="junk")
                gidx = wpool.tile([B, 1], FP, tag="gidx")
                nc.vector.tensor_tensor(out=junk[:, :], in0=mskc[:, :],
                                        in1=rr[:, :, 1], op=mybir.AluOpType.mult)
                nc.vector.tensor_reduce(
                    out=gidx[:, :], in_=junk[:, :], axis=mybir.AxisListType.X,
                    op=mybir.AluOpType.add,
                )
                idxi = wpool.tile([B, 1], mybir.dt.int32, tag="idxi")
                nc.vector.tensor_copy(out=idxi[:, :], in_=gidx[:, :])
                if debug:
                    dstep = dbg_dram[t:t + 1, :]
                    nc.scalar.dma_start(out=dstep[:, 0:64], in_=bm[0:1, :])
                    nc.scalar.dma_start(out=dstep[:, 64:192],
                                        in_=AP(xin[:, :].tensor, 0, [[64, 128], [1, 1]]))
                    nc.scalar.dma_start(out=dstep[:, 192:320],
                                        in_=AP(xin[:, :].tensor, 32, [[64, 128], [1, 1]]))
                    nc.scalar.dma_start(
                        out=dstep[:, 320:832],
                        in_=AP(rcvb[:, :].tensor, 0, [[16, 32], [1, 16]]))
                    nc.scalar.dma_start(out=dstep[:, 832:864],
                                        in_=AP(rv[:, :].tensor, 0, [[1, 32], [1, 1]]))
                    nc.scalar.dma_start(out=dstep[:, 864:896],
                                        in_=AP(gidx[:, :].tensor, 0, [[1, 32], [1, 1]]))

                # ---------- gather G'[tok] ----------
                xg = wpool.tile([B, G3], FP, tag="xg")
                nc.gpsimd.indirect_dma_start(
                    out=xg[:, :], out_offset=None,
                    in_=gp_in[:, :],
                    in_offset=IndirectOffsetOnAxis(ap=idxi[:, 0:1], axis=0),
                    **(dict(bounds_check=V - 1, oob_is_err=False) if bc else {}),
                )
                # transpose gi gates into separate 512B-aligned psum slots
                nc.tensor.matmul(
                    out=pt[:, 512:544], lhsT=xg[:, 0:16], rhs=ident_sb[0:B, 0:B],
                    start=True, stop=True, is_transpose=True,
                )
                nc.tensor.matmul(
                    out=pt[:, 640:672], lhsT=xg[:, 16:32], rhs=ident_sb[0:B, 0:B],
                    start=True, stop=True, is_transpose=True,
                )
                nc.tensor.matmul(
                    out=pt[:, 384:416], lhsT=xg[:, 32:48], rhs=ident_sb[0:B, 0:B],
                    start=True, stop=True, is_transpose=True,
                )

                # ---------- GRU (all [16, 32] at partitions 0:16) ----------
                rzsum = wpool.tile([H, 64], FP, tag="rzsum")
                nc.vector.tensor_add(out=rzsum[:, 0:32], in0=ghsb[:, 0:32],
                                     in1=pt[:, 512:544])
                nc.vector.tensor_add(out=rzsum[:, 32:64], in0=ghsb[:, 32:64],
                                     in1=pt[:, 640:672])
                rzsig = wpool.tile([H, 64], FP, tag="rzsig")
                nc.scalar.activation(out=rzsig[:, :], in_=rzsum[:, :],
                                     func=mybir.ActivationFunctionType.Sigmoid)
                rh = wpool.tile([H, B], FP, tag="rh")
                nc.vector.tensor_mul(out=rh[:, :], in0=rzsig[:, 0:32],
                                     in1=pt[:, 256:288])
                ns_ = wpool.tile([H, B], FP, tag="ns")
                nc.vector.tensor_add(out=ns_[:, :], in0=rh[:, :], in1=pt[:, 384:416])
                nn_ = wpool.tile([H, B], FP, tag="nn")
                nc.scalar.activation(out=nn_[:, :], in_=ns_[:, :],
                                     func=mybir.ActivationFunctionType.Tanh)
                dd = wpool.tile([H, B], FP, tag="dd")
                nc.vector.tensor_sub(out=dd[:, :], in0=hT_sb[0:H, :], in1=nn_[:, :])
                zd = wpool.tile([H, B], FP, tag="zd")
                nc.vector.tensor_mul(out=zd[:, :], in0=rzsig[:, 32:64], in1=dd[:, :])
                hTw = nc.vector.tensor_add(out=hT_sb[0:H, :], in0=nn_[:, :], in1=zd[:, :])
                if debug:
                    nc.scalar.dma_start(out=dstep[:, 896:896 + 32 * 48],
                                        in_=AP(xg[:, :].tensor, 0, [[48, 32], [1, 48]]))
                    nc.scalar.dma_start(out=dstep[:, 2432:2432 + 17 * 32],
                                        in_=AP(hT_sb[:, :].tensor, 0, [[32, 17], [1, 32]]))
                    ptc = wpool.tile([H, 224], FP, tag="ptdbg")
                    nc.vector.memset(ptc[:, :], 0.0)
                    for si, so in ((0, 0), (128, 32), (256, 64), (384, 96),
                                   (512, 128), (640, 160)):
                        nc.vector.tensor_copy(out=ptc[:, so:so + 32],
                                              in_=pt[:, si:si + 32])
                    nc.scalar.dma_start(out=dstep[:, 2976:2976 + 16 * 224],
                                        in_=AP(ptc[:, :].tensor, 0, [[224, 16], [1, 224]]))

                # ---------- output DMA ----------
                if t % 4 == 3 or t == n_steps - 1:
                    nt = t % 4 + 1
                    t0_ = t - nt + 1
                    for tt in range(nt):
                        dst = AP(out_dram, ((t0_ + tt) % out_steps) * VLOC,
                                 [[1000, 4], [out_steps * VLOC, B], [1, F]])
                        nc.scalar.dma_start(
                            out=dst, in_=slab[:, tt * FPAD:tt * FPAD + F])

    for bi, sem, thr in patches:
        bi.wait_op(sem, thr, "sem-ge")
    nc.finalize()
    return nc


def host_prep(inputs, n_steps=T):
    """Build per-core input maps from the full problem inputs."""
    emb = np.asarray(inputs["embedding"], np.float32)
    W_ih = np.asarray(inputs["W_ih"], np.float32)
    W_hh = np.asarray(inputs["W_hh"], np.float32)
    b_ih = np.asarray(inputs["b_ih"], np.float32)
    b_hh = np.asarray(inputs["b_hh"], np.float32)
    W_out = np.asarray(inputs["W_out"], np.float32)
    b_out = np.asarray(inputs["b_out"], np.float32)
    h0 = np.asarray(inputs["encoder_hidden"], np.float32)[0]  # [B, H]

    # G' = emb @ W_ih.T + b_ih  [V, 48]
    gp = (emb @ W_ih.T + b_ih).astype(np.float32)
    # W_aug [V, 17]
    w_aug = np.concatenate([W_out, b_out[:, None]], axis=1).astype(np.float32)
    # whh_aug.T [17, 48]
    whh = np.concatenate([W_hh.T, b_hh[None, :]], axis=0).astype(np.float32)

    # The kernel's iteration t computes logits_t from its current state, so the
    # initial state must be h1 = GRU(emb[SOS=0], h0), computed here in fp32.
    x0 = np.broadcast_to(emb[0], (B, E))
    gi = (x0 @ W_ih.T + b_ih).astype(np.float32)
    gh = (h0 @ W_hh.T + b_hh).astype(np.float32)
    i_r, i_z, i_n = gi[:, :H], gi[:, H:2 * H], gi[:, 2 * H:]
    h_r, h_z, h_n = gh[:, :H], gh[:, H:2 * H], gh[:, 2 * H:]
    r = (1.0 / (1.0 + np.exp(-(i_r + h_r), dtype=np.float32))).astype(np.float32)
    z = (1.0 / (1.0 + np.exp(-(i_z + h_z), dtype=np.float32))).astype(np.float32)
    n = np.tanh(i_n + r * h_n, dtype=np.float32).astype(np.float32)
    h1 = ((1.0 - z) * n + z * h0).astype(np.float32)

    h0a = np.concatenate([h1, np.ones((B, 1), np.float32)], axis=1)  # [32, 17]
    h0t = h0a.T.copy()                                               # [17, 32]

    ident = np.eye(128, dtype=np.float32)

    in_maps = []
    for c in range(NCORES):
        wt = np.zeros((K, 4 * FPAD), np.float32)
        for j in range(4):
            blk = np.zeros((K, FPAD), np.float32)
            blk[K - 1, :] = -1.0e30          # pad slots: bias -inf
            v0 = c * VLOC + j * 1000
            blk[:, 0:F] = w_aug[v0:v0 + F, :].T
            wt[:, j * FPAD:(j + 1) * FPAD] = blk
        off2 = np.zeros((128, 1), np.float32)
        for j in range(4):
            off2[32 * j:32 * j + 32, 0] = c * VLOC + j * 1000
        in_maps.append({
            "wt": wt, "whh": whh, "h0t": h0t, "h0a": h0a,
            "off2": off2, "ident": ident, "gp": gp,
        })
    return in_maps


def assemble_output(results, n_steps=T):
    """Concatenate per-core [B, T, VLOC] stripes into [B, T, V]."""
    return np.concatenate([r["out"] for r in results], axis=2)


_NC_CACHE = {}

# Best validated configuration for the kernel() entrypoint:
#   "v1":     single-chain, allgather exchange   (~35 us/step)
#   "v4n8":   8-way interleaved, fp32 scan
#   "v4n8bf": 8-way interleaved, bf16-pair scan
BEST = "v1"


def _build_best(n_steps=T, reps=1, out_steps=None):
    if BEST == "v1":
        return build_nc(n_steps=n_steps, exchange="allgather", reps=reps,
                        out_steps=out_steps)
    if BEST == "v4n8":
        return build_nc_v4(n_steps=n_steps, nch=8, reps=reps,
                           out_steps=out_steps)
    if BEST == "v4n8bf":
        return build_nc_v4(n_steps=n_steps, nch=8, reps=reps,
                           out_steps=out_steps, bf16pair=True)
    raise ValueError(BEST)


def _prep_best(inputs):
    if BEST == "v1":
        return host_prep(inputs)
    return host_prep_v3(inputs, nch=8, bf16pair=BEST.endswith("bf"))


def kernel(**inputs):
    """Full-input entrypoint: shard across 8 NeuronCores, run the Bass kernel,
    return the full (32, 100, 32000) float32 logits tensor."""
    from concourse.bass_utils import run_bass_kernel_spmd

    key = (BEST, T)
    if key not in _NC_CACHE:
        _NC_CACHE[key] = _build_best(n_steps=T)
    nc = _NC_CACHE[key]
    in_maps = _prep_best(inputs)
    res = run_bass_kernel_spmd(nc, in_maps, core_ids=list(range(NCORES)))
    return assemble_output(res.results)


# ---------------------------------------------------------------------------
# v3: N-way batch-interleaved chains; each chain's collective exchange is in
# flight while the other chains compute, hiding the ~28us collective latency.
# Slot s advances chain c = s % N by one step. Per chain-step shapes:
#   batches BCH = 32/N; vocab groups G = 4N (4 PE col positions x N k-offsets
#   packed via zero-padded accumulating matmuls); PSUM scan tile [128, FG].
# ---------------------------------------------------------------------------

def build_nc_v3(n_steps=T, nch=4, reps=1, out_steps=None, bf16pair=False):
    if out_steps is None:
        out_steps = n_steps
    BCH = B // nch              # batches per chain
    PPOS = 32 // BCH            # k-offsets per column position (= nch)
    G = 4 * PPOS                # vocab groups
    VG = VLOC // G              # valid vocab per group
    FG = 1 << (VG - 1).bit_length()  # padded free size per group
    PW = 32 * PPOS              # padded stationary width per chain
    KS = 32 + BCH               # stride of h-position across k blocks

    nc = bacc.Bacc(None, target_bir_lowering=False)

    BF16 = mybir.dt.bfloat16
    if bf16pair:
        wth_in = nc.dram_tensor("wt3h", [K, G * FG], BF16, kind="ExternalInput")
        wtl_in = nc.dram_tensor("wt3l", [K, G * FG], BF16, kind="ExternalInput")
        hph_in = nc.dram_tensor("hp0h", [K, nch * PW], BF16, kind="ExternalInput")
        hpl_in = nc.dram_tensor("hp0l", [K, nch * PW], BF16, kind="ExternalInput")
    else:
        wt_in = nc.dram_tensor("wt3", [K, G * FG], FP, kind="ExternalInput")
    whh_in = nc.dram_tensor("whh", [K, G3], FP, kind="ExternalInput")
    hp0_in = nc.dram_tensor("hp0", [K, nch * PW], FP, kind="ExternalInput")
    off3_in = nc.dram_tensor("off3", [128, 1], FP, kind="ExternalInput")
    ident_in = nc.dram_tensor("ident", [128, 128], FP, kind="ExternalInput")
    gp_in = nc.dram_tensor("gp", [V, G3], FP, kind="ExternalInput")
    out_dram = nc.dram_tensor("out", [B, out_steps, VLOC], FP, kind="ExternalOutput")

    with tile.TileContext(nc) as tc:
        with (
            tc.tile_pool(name="const", bufs=1) as cpool,
            tc.tile_pool(name="state", bufs=1) as spool,
            tc.tile_pool(name="work", bufs=4) as wpool,
            tc.tile_pool(name="stage", bufs=4) as stpool,
            tc.tile_pool(name="psumL", bufs=2, space="PSUM") as plpool,
            tc.tile_pool(name="psumG", bufs=2, space="PSUM") as pgpool,
            tc.tile_pool(name="psumT", bufs=2, space="PSUM") as ptpool,
            tc.tile_pool(name="dram", bufs=2 * nch + 2, space="DRAM") as dpool,
        ):
            if bf16pair:
                wth_sb = cpool.tile([K, G * FG], BF16, tag="wth")
                nc.gpsimd.dma_start(out=wth_sb[:, :], in_=wth_in[:, :])
                wtl_sb = cpool.tile([K, G * FG], BF16, tag="wtl")
                nc.gpsimd.dma_start(out=wtl_sb[:, :], in_=wtl_in[:, :])
            else:
                wt_sb = cpool.tile([K, G * FG], FP, tag="wt")
                nc.gpsimd.dma_start(out=wt_sb[:, :], in_=wt_in[:, :])
            whh_sb = cpool.tile([K, G3], FP, tag="whh")
            nc.gpsimd.dma_start(out=whh_sb[:, :], in_=whh_in[:, :])
            off3_sb = cpool.tile([128, 1], FP, tag="off3")
            nc.gpsimd.dma_start(out=off3_sb[:, :], in_=off3_in[:, :])
            ident_sb = cpool.tile([128, 128], FP, tag="ident")
            nc.gpsimd.dma_start(out=ident_sb[:, :], in_=ident_in[:, :])

            # per-chain zero-padded stationary state [17, PW]
            hp = []
            for c in range(nch):
                t_ = spool.tile([K, PW], FP, tag=f"hp{c}")
                nc.gpsimd.dma_start(out=t_[:, :], in_=hp0_in[:, c * PW:(c + 1) * PW])
                hp.append(t_)
            if bf16pair:
                hph, hpl = [], []
                for c in range(nch):
                    th = spool.tile([K, PW], BF16, tag=f"hph{c}")
                    nc.gpsimd.dma_start(out=th[:, :], in_=hph_in[:, c * PW:(c + 1) * PW])
                    hph.append(th)
                    tl = spool.tile([K, PW], BF16, tag=f"hpl{c}")
                    nc.gpsimd.dma_start(out=tl[:, :], in_=hpl_in[:, c * PW:(c + 1) * PW])
                    hpl.append(tl)

            bm = spool.tile([B, 64], FP, tag="bm")
            nc.vector.memset(bm[:, :], 0.0)
            xin = spool.tile([128, 64], FP, tag="xin")
            nc.vector.memset(xin[:, :], 0.0)

            pend = {}  # chain -> cc_out tile of its in-flight exchange

            for rep in range(reps):
              for s in range(nch * n_steps):
                c = s % nch
                t = s // nch
                hpc = hp[c]

                if t >= 1:
                    # ---- gh matmuls from h_{t-1} (needed by GRU below) ----
                    pt = pgpool.tile([H, 768], FP, tag="pt")
                    for g in range(3):
                        nc.tensor.matmul(
                            out=pt[:, 128 * g:128 * g + BCH],
                            lhsT=whh_sb[:, 16 * g:16 * g + 16],
                            rhs=hpc[:, 0:BCH],
                            start=True, stop=True, tile_position=(0, 0),
                        )
                    ghsb = wpool.tile([H, 2 * BCH], FP, tag="ghsb")
                    nc.scalar.copy(
                        out=ghsb[:, :].rearrange("p (x f) -> p x f", x=2),
                        in_=pt[:, 0:256].rearrange(
                            "p (x f) -> p x f", x=2)[:, :, 0:BCH])

                    # ---- consume pending exchange of step t-1 ----
                    cc_out = pend.pop(c)
                    rcvb = wpool.tile([BCH, 2 * NCORES], FP, tag="rcvb")
                    nc.scalar.dma_start(
                        out=rcvb[:, :],
                        in_=AP(cc_out[:, :].tensor, 0,
                               [[2, BCH], [128 * 2, NCORES], [1, 2]]),
                    )
                    rr = rcvb[:, :].rearrange("p (c x) -> p c x", x=2)
                    rv = wpool.tile([BCH, 1], FP, tag="rv")
                    nc.vector.tensor_reduce(
                        out=rv[:, :], in_=rr[:, :, 0],
                        axis=mybir.AxisListType.X, op=mybir.AluOpType.max,
                    )
                    mskc = wpool.tile([BCH, NCORES], FP, tag="mskc")
                    nc.vector.tensor_scalar(
                        out=mskc[:, :], in0=rr[:, :, 0],
                        scalar1=rv[:, 0:1], scalar2=None,
                        op0=mybir.AluOpType.is_equal,
                    )
                    junk = wpool.tile([BCH, NCORES], FP, tag="junk")
                    nc.vector.tensor_tensor(out=junk[:, :], in0=mskc[:, :],
                                            in1=rr[:, :, 1],
                                            op=mybir.AluOpType.mult)
                    gidx = wpool.tile([BCH, 1], FP, tag="gidx")
                    nc.vector.tensor_reduce(
                        out=gidx[:, :], in_=junk[:, :],
                        axis=mybir.AxisListType.X, op=mybir.AluOpType.add,
                    )
                    idxi = wpool.tile([BCH, 1], mybir.dt.int32, tag="idxi")
                    nc.vector.tensor_copy(out=idxi[:, :], in_=gidx[:, :])

                    xg = wpool.tile([BCH, G3], FP, tag="xg")
                    nc.gpsimd.indirect_dma_start(
                        out=xg[:, :], out_offset=None,
                        in_=gp_in[:, :],
                        in_offset=IndirectOffsetOnAxis(ap=idxi[:, 0:1], axis=0),
                    )
                    # gi gate transposes into 512B-aligned psum slots
                    nc.tensor.matmul(
                        out=pt[:, 512:512 + BCH], lhsT=xg[:, 0:16],
                        rhs=ident_sb[0:BCH, 0:BCH],
                        start=True, stop=True, is_transpose=True,
                    )
                    nc.tensor.matmul(
                        out=pt[:, 640:640 + BCH], lhsT=xg[:, 16:32],
                        rhs=ident_sb[0:BCH, 0:BCH],
                        start=True, stop=True, is_transpose=True,
                    )
                    nc.tensor.matmul(
                        out=pt[:, 384:384 + BCH], lhsT=xg[:, 32:48],
                        rhs=ident_sb[0:BCH, 0:BCH],
                        start=True, stop=True, is_transpose=True,
                    )

                    # ---- GRU ----
                    rzsum = wpool.tile([H, 2 * BCH], FP, tag="rzsum")
                    nc.vector.tensor_add(
                        out=rzsum[:, :].rearrange("p (x f) -> p x f", x=2),
                        in0=ghsb[:, :].rearrange("p (x f) -> p x f", x=2),
                        in1=pt[:, 512:768].rearrange(
                            "p (x f) -> p x f", x=2)[:, :, 0:BCH])
                    rzsig = wpool.tile([H, 2 * BCH], FP, tag="rzsig")
                    nc.scalar.activation(out=rzsig[:, :], in_=rzsum[:, :],
                                         func=mybir.ActivationFunctionType.Sigmoid)
                    rh = wpool.tile([H, BCH], FP, tag="rh")
                    nc.vector.tensor_mul(out=rh[:, :], in0=rzsig[:, 0:BCH],
                                         in1=pt[:, 256:256 + BCH])
                    ns_ = wpool.tile([H, BCH], FP, tag="ns")
                    nc.vector.tensor_add(out=ns_[:, :], in0=rh[:, :],
                                         in1=pt[:, 384:384 + BCH])
                    nn_ = wpool.tile([H, BCH], FP, tag="nn")
                    nc.scalar.activation(out=nn_[:, :], in_=ns_[:, :],
                                         func=mybir.ActivationFunctionType.Tanh)
                    dd = wpool.tile([H, BCH], FP, tag="dd")
                    nc.vector.tensor_sub(out=dd[:, :], in0=hpc[0:H, 0:BCH],
                                         in1=nn_[:, :])
                    zd = wpool.tile([H, BCH], FP, tag="zd")
                    nc.vector.tensor_mul(out=zd[:, :], in0=rzsig[:, BCH:2 * BCH],
                                         in1=dd[:, :])
                    hn = wpool.tile([H, BCH], FP, tag="hn")
                    nc.vector.tensor_add(out=hn[:, :], in0=nn_[:, :], in1=zd[:, :])
                    # scatter h_t into all k-offset blocks of hpc
                    nc.vector.tensor_copy(
                        out=AP(hpc[:, :].tensor, 0,
                               [[PW, H], [KS, PPOS], [1, BCH]]),
                        in_=hn[:, :].unsqueeze(1).to_broadcast([H, PPOS, BCH]),
                    )
                    if bf16pair:
                        nc.vector.tensor_copy(
                            out=AP(hph[c][:, :].tensor, 0,
                                   [[PW, H], [KS, PPOS], [1, BCH]]),
                            in_=hn[:, :].unsqueeze(1).to_broadcast([H, PPOS, BCH]),
                        )
                        hlo = wpool.tile([H, BCH], FP, tag="hlo")
                        nc.vector.tensor_sub(out=hlo[:, :], in0=hn[:, :],
                                             in1=hph[c][0:H, 0:BCH])
                        nc.vector.tensor_copy(
                            out=AP(hpl[c][:, :].tensor, 0,
                                   [[PW, H], [KS, PPOS], [1, BCH]]),
                            in_=hlo[:, :].unsqueeze(1).to_broadcast([H, PPOS, BCH]),
                        )

                # ---- scan: G accumulating matmuls ----
                lps = plpool.tile([128, FG], FP, tag="L")
                for j in range(4):
                    for k in range(PPOS):
                        g = PPOS * j + k
                        if bf16pair:
                            terms = [(hph[c], wth_sb), (hpl[c], wth_sb),
                                     (hph[c], wtl_sb)]
                            for ti, (hs, ws) in enumerate(terms):
                                nc.tensor.matmul(
                                    out=lps[32 * j:32 * j + 32, :],
                                    lhsT=hs[:, 32 * k:32 * k + 32],
                                    rhs=ws[:, g * FG:(g + 1) * FG],
                                    start=(k == 0 and ti == 0),
                                    stop=(k == PPOS - 1 and ti == 2),
                                    tile_position=(0, 32 * j),
                                )
                        else:
                            nc.tensor.matmul(
                                out=lps[32 * j:32 * j + 32, :],
                                lhsT=hpc[:, 32 * k:32 * k + 32],
                                rhs=wt_sb[:, g * FG:(g + 1) * FG],
                                start=(k == 0), stop=(k == PPOS - 1),
                                tile_position=(0, 32 * j),
                            )

                # ---- stage + output ----
                stg = stpool.tile([128, FG], FP, tag="stg")
                nc.scalar.copy(out=stg[:, :], in_=lps[:, :])
                dst = AP(out_dram,
                         (c * BCH) * (out_steps * VLOC) + (t % out_steps) * VLOC,
                         [[VG * PPOS, 4], [VG, PPOS],
                          [out_steps * VLOC, BCH], [1, VG]])
                nc.scalar.dma_start(out=dst, in_=stg[:, 0:VG])

                if t <= n_steps - 2:
                    # ---- local argmax ----
                    mx8 = wpool.tile([128, 8], FP, tag="mx8")
                    nc.vector.max(out=mx8[:, :], in_=lps[:, :])
                    mi8 = wpool.tile([128, 8], mybir.dt.uint32, tag="mi8")
                    nc.vector.max_index(out=mi8[:, :], in_max=mx8[:, :],
                                        in_values=lps[:, :])
                    cwi = wpool.tile([128, 1], FP, tag="cwi")
                    nc.vector.tensor_scalar_add(cwi[:, :], mi8[:, 0:1],
                                                off3_sb[:, 0:1])
                    tj = ptpool.tile([1, 256], FP, tag="tj")
                    tjv = tj[:, 0:128]
                    tji = tj[:, 128:256]
                    nc.tensor.transpose(out=tjv, in_=mx8[:, 0:1],
                                        identity=ident_sb[:, :])


# revision 12
# speedup vs baseline: 1.0787x; 1.0787x over previous
"""GRU greedy decoder on 8 trn2 cores.

Vocab-sharded: each core owns 4000 vocab entries of the out-projection; per
step it computes its local (B=32, 4000) logits, finds the local argmax, all
cores exchange (max-value, global-index) candidates, everyone computes the
global argmax, gathers the fused embedding row G'[tok] = emb[tok]@W_ih.T+b_ih,
and advances the replicated GRU state. Logits are staged to SBUF and DMA'd to
each core's output stripe.

Layout:
  PSUM logits tile [128, 1024]: partition 32j + m (j = psum col group, m =
  batch), free = pos in [0, 1024); vocab v = core*4000 + j*1000 + pos for
  pos < 1000; pos in [1000, 1024) are pad slots with bias -1e30.
  Matmuls: col-group tiling only (row groups != 0 crash at runtime on this
  stack): per j, two N=512 matmuls; lhsT = hT_aug [17, 32] at partitions 0:17.
"""

import numpy as np
import concourse.bass as bass
import concourse.bacc as bacc
import concourse.mybir as mybir
from concourse import tile
from concourse.bass import AP, IndirectOffsetOnAxis
from concourse.tile_rust import add_dep_helper

FP = mybir.dt.float32
B, H, E, V, T = 32, 16, 16, 32000, 100
NCORES = 8
VLOC = V // NCORES           # 4000
F = VLOC // 4                # 1000 valid entries per partition
FPAD = 1024                  # padded free size (PSUM bank alignment)
K = H + 1                    # 17: h dims + ones row (bias)
G3 = 3 * H                   # 48


def build_nc(n_steps=T, exchange="allgather", psum_max=True, reps=1,
             out_steps=None, mm_dtype=FP, xdma_pool=False):
    # out_steps: number of distinct output step-slots (timing runs use a small
    # value to shrink D2H transfer; DMA count/size per step is unchanged).
    # mm_dtype: dtype the big vocab matmul operands are bitcast to (FP or
    # float32r for the 1-cycle/row PE mode).
    if out_steps is None:
        out_steps = n_steps
    nc = bacc.Bacc(None, target_bir_lowering=False)

    # ---- I/O ----
    wt_in = nc.dram_tensor("wt", [K, 4 * FPAD], FP, kind="ExternalInput")
    whh_in = nc.dram_tensor("whh", [K, G3], FP, kind="ExternalInput")
    h0t_in = nc.dram_tensor("h0t", [K, B], FP, kind="ExternalInput")
    h0a_in = nc.dram_tensor("h0a", [B, K], FP, kind="ExternalInput")
    off2_in = nc.dram_tensor("off2", [128, 1], FP, kind="ExternalInput")
    ident_in = nc.dram_tensor("ident", [128, 128], FP, kind="ExternalInput")
    gp_in = nc.dram_tensor("gp", [V, G3], FP, kind="ExternalInput")
    out_dram = nc.dram_tensor("out", [B, out_steps, VLOC], FP, kind="ExternalOutput")

    with tile.TileContext(nc) as tc:
        with (
            tc.tile_pool(name="const", bufs=1) as cpool,
            tc.tile_pool(name="state", bufs=1) as spool,
            tc.tile_pool(name="work", bufs=4) as wpool,
            tc.tile_pool(name="stage", bufs=3) as stpool,
            tc.tile_pool(name="psumL", bufs=2, space="PSUM") as plpool,
            tc.tile_pool(name="psumG", bufs=1, space="PSUM") as pgpool,
            tc.tile_pool(name="psumT", bufs=1, space="PSUM") as ptpool,
            tc.tile_pool(name="dram", bufs=4, space="DRAM") as dpool,
        ):
            # ---- constants to SBUF ----
            wt_sb = cpool.tile([K, 4 * FPAD], FP, tag="wt")
            nc.gpsimd.dma_start(out=wt_sb[:, :], in_=wt_in[:, :])
            whh_sb = cpool.tile([K, G3], FP, tag="whh")
            nc.gpsimd.dma_start(out=whh_sb[:, :], in_=whh_in[:, :])
            off2_sb = cpool.tile([128, 1], FP, tag="off2")
            nc.gpsimd.dma_start(out=off2_sb[:, :], in_=off2_in[:, :])
            ident_sb = cpool.tile([128, 128], FP, tag="ident")
            nc.gpsimd.dma_start(out=ident_sb[:, :], in_=ident_in[:, :])

            # ---- state ----
            hT_sb = spool.tile([K, B], FP, tag="hT")     # hT_aug [17, 32]
            nc.gpsimd.dma_start(out=hT_sb[:, :], in_=h0t_in[:, :])
            if mm_dtype != FP:
                # one-time rounding of scan weights to f32r; per-step h round
                wt_r = cpool.tile([K, 4 * FPAD], mm_dtype, tag="wtr")
                nc.vector.tensor_copy(out=wt_r[:, :], in_=wt_sb[:, :])
            ha_sb = spool.tile([B, K], FP, tag="ha")     # h_aug [32, 17] col 16 = ones
            nc.gpsimd.dma_start(out=ha_sb[:, :], in_=h0a_in[:, :])
            vs32 = spool.tile([B, B], FP, tag="vs32")    # row 0 = selected idx
            nc.vector.memset(vs32[:, :], 0.0)

            RING = 4
            patches = []   # (BassInstruction, sem, threshold) applied post-schedule
            if exchange == "remote":
                rsem = nc.alloc_semaphore("rsem")
                lsem = nc.alloc_semaphore("lsem")
                rcvb = spool.tile([128, RING * 16], FP, tag="rcvb")
                pid = nc.gpsimd.partition_id()
                poff = pid * 2
                # Launch barrier: cores may start the NEFF skewed; a remote
                # send arriving before the receiver zeroed its semaphores is
                # lost -> deadlock. A dummy AllGather completes only once all
                # cores are running; its (zero) result is added to hT so no
                # step-0 compute (hence no remote send) precedes it.
                barsrc = wpool.tile([128, 1], FP, tag="barsrc")
                nc.vector.memset(barsrc[:, :], 0.0)
                bar_in = dpool.tile([128, 1], FP, tag="barin")
                bar_out = dpool.tile([128 * NCORES, 1], FP, tag="barout")
                nc.scalar.dma_start(out=bar_in[:, :], in_=barsrc[:, :])
                nc.gpsimd.collective_compute(
                    "AllGather",
                    mybir.AluOpType.bypass,
                    ins=[bar_in[:, :].opt()],
                    outs=[bar_out[:, :].opt()],
                    replica_groups=[list(range(NCORES))],
                )
                barz = spool.tile([K, 1], FP, tag="barz")
                nc.scalar.dma_start(out=barz[:, :], in_=bar_out[0:K, 0:1])
                nc.vector.tensor_scalar_add(hT_sb[:, :], hT_sb[:, :], barz[:, 0:1])

            for rep in range(reps):
              for t in range(n_steps):
                tg = rep * n_steps + t
                # ---------- logits matmuls (col-group tiling) ----------
                if mm_dtype != FP:
                    hT_r = wpool.tile([K, B], mm_dtype, tag="hTr")
                    nc.vector.tensor_copy(out=hT_r[:, :], in_=hT_sb[:, :])
                    scan_lhs, scan_rhs = hT_r, wt_r
                else:
                    scan_lhs, scan_rhs = hT_sb, wt_sb
                lps = plpool.tile([128, FPAD], FP, tag="L")
                for j in range(4):
                    for hh in range(2):
                        nc.tensor.matmul(
                            out=lps[32 * j:32 * j + B, hh * 512:(hh + 1) * 512],
                            lhsT=scan_lhs[:, :],
                            rhs=scan_rhs[:, j * FPAD + hh * 512:j * FPAD + (hh + 1) * 512],
                            start=True, stop=True,
                            tile_position=(0, 32 * j),
                        )
                # gh = h @ W_hh.T + b_hh  -> [32, 48]
                ghp = pgpool.tile([B, G3], FP, tag="gh")
                nc.tensor.matmul(
                    out=ghp[:, :], lhsT=hT_sb[:, :], rhs=whh_sb[:, :],
                    start=True, stop=True, tile_position=(0, 0),
                )

                # ---------- stage to SBUF for output (4-step slabs) ----------
                if t % 4 == 0:
                    slab = stpool.tile([128, 4 * FPAD], FP, tag="stg4")
                stg = slab[:, (t % 4) * FPAD:(t % 4 + 1) * FPAD]
                nc.scalar.copy(out=stg, in_=lps[:, :])

                # ---------- local argmax ----------
                mx8 = wpool.tile([128, 8], FP, tag="mx8")
                mi8 = wpool.tile([128, 8], mybir.dt.uint32, tag="mi8")
                src = lps
                nc.vector.max(out=mx8[:, :], in_=src[:, :])
                nc.vector.max_index(out=mi8[:, :], in_max=mx8[:, :], in_values=src[:, :])

                # payload CW [128, 2] = (val, global idx as fp32)
                cw = wpool.tile([128, 2], FP, tag="cw")
                if exchange == "remote" and tg >= 3:
                    wn = nc.vector.nop(nofuse=True, hint="lsem_wait")
                    patches.append((wn, lsem, 16 * (tg - 2)))
                cwi1 = nc.vector.tensor_copy(out=cw[:, 0:1], in_=mx8[:, 0:1])
                cwi2 = nc.vector.tensor_scalar_add(cw[:, 1:2], mi8[:, 0:1], off2_sb[:, 0:1])
                if exchange == "remote" and tg >= 3:
                    add_dep_helper(wn.ins, cwi1.ins, sync=False, reason="lsem order")
                    add_dep_helper(wn.ins, cwi2.ins, sync=False, reason="lsem order")

                # ---------- exchange ----------
                if exchange.startswith("allgather"):
                    xeng = nc.gpsimd if xdma_pool else nc.scalar
                    cc_in = dpool.tile([128, 2], FP, tag="ccin")
                    cc_out = dpool.tile([128 * NCORES, 2], FP, tag="ccout")
                    xeng.dma_start(out=cc_in[:, :], in_=cw[:, :])
                    nc.gpsimd.collective_compute(
                        "AllGather",
                        mybir.AluOpType.bypass,
                        ins=[cc_in[:, :].opt()],
                        outs=[cc_out[:, :].opt()],
                        replica_groups=[list(range(NCORES))],
                    )
                    if exchange == "allgather2":
                        cc_out2 = dpool.tile([128 * NCORES, 2], FP, tag="ccout2")
                        nc.gpsimd.collective_compute(
                            "AllGather",
                            mybir.AluOpType.bypass,
                            ins=[cc_in[:, :].opt()],
                            outs=[cc_out2[:, :].opt()],
                            replica_groups=[list(range(NCORES))],
                        )
                    rcv = wpool.tile([128, NCORES * 2], FP, tag="rcv")
                    # iterate (p, c, x): strides in elements: p->2, c->256, x->1
                    xeng.dma_start(
                        out=rcv[:, :],
                        in_=AP(cc_out[:, :].tensor, 0,
                               [[2, 128], [128 * 2, NCORES], [1, 2]]),
                    )
                elif exchange == "remote":
                    slot = t % RING
                    out_ap = AP(rcvb[:, :].tensor, slot * 16 + poff,
                                [[RING * 16, 128], [1, 2]])
                    nc.gpsimd.remote_dma_broadcast(
                        out_ap=out_ap, in_ap=cw[:, :],
                        remote_sem=rsem, local_sem=lsem,
                        rdests=[(0, k) for k in range(NCORES)],
                    )
                    nc.gpsimd.trigger_dma(count=None)
                    rcv = rcvb[:, slot * 16:(slot + 1) * 16]
                elif exchange == "none":
                    rcv = None
                else:
                    raise ValueError(exchange)

                cw2 = wpool.tile([128, 2], FP, tag="cw2")
                if rcv is not None and exchange in ("allgather", "remote"):
                    # core-combine: best over 8 cores per (j, m) partition
                    rm8 = wpool.tile([128, 8], FP, tag="rm8")
                    if exchange == "remote":
                        rr = rcv.rearrange("p (c x) -> p c x", x=2)
                    else:
                        rr = rcv[:, :].rearrange("p (c x) -> p c x", x=2)
                    vals = rr[:, :, 0]
                    idxs = rr[:, :, 1]
                    if exchange == "remote":
                        wr = nc.vector.nop(nofuse=True, hint="rsem_wait")
                        patches.append((wr, rsem, 16 * (tg + 1)))
                    mxi = nc.vector.max(out=rm8[:, :], in_=vals)
                    if exchange == "remote":
                        add_dep_helper(wr.ins, mxi.ins, sync=False, reason="rsem order")
                    mskc = wpool.tile([128, 8], FP, tag="mskc")
                    nc.vector.tensor_scalar(
                        out=mskc[:, :], in0=vals, scalar1=rm8[:, 0:1], scalar2=None,
                        op0=mybir.AluOpType.is_equal,
                    )
                    tmpc = wpool.tile([128, 8], FP, tag="tmpc")
                    nc.vector.tensor_tensor(out=tmpc[:, :], in0=mskc[:, :], in1=idxs,
                                            op=mybir.AluOpType.mult)
                    gidxc = wpool.tile([128, 1], FP, tag="gidxc")
                    nc.vector.tensor_reduce(
                        out=gidxc[:, :], in_=tmpc[:, :], axis=mybir.AxisListType.X,
                        op=mybir.AluOpType.add,
                    )
                    cwv, cwi = rm8[:, 0:1], gidxc[:, :]
                else:
                    if rcv is not None:
                        nc.vector.tensor_copy(out=cw2[:, :], in_=rcv[:, 0:2])
                    else:
                        nc.vector.tensor_copy(out=cw2[:, :], in_=cw[:, :])
                    cwv, cwi = cw2[:, 0:1], cw2[:, 1:2]

                # ---------- j-combine ----------
                tj = ptpool.tile([1, 256], FP, tag="tj")
                tjv = tj[:, 0:128]
                tji = tj[:, 128:256]
                nc.tensor.transpose(out=tjv, in_=cwv, identity=ident_sb[:, :])
                nc.tensor.transpose(out=tji, in_=cwi, identity=ident_sb[:, :])
                gmj = wpool.tile([1, B], FP, tag="gmj")
                vrow = tjv.rearrange("p (j m) -> p m j", j=4)
                irow = tji.rearrange("p (j m) -> p m j", j=4)
                nc.vector.tensor_reduce(
                    out=gmj[:, :], in_=vrow, axis=mybir.AxisListType.X,
                    op=mybir.AluOpType.max,
                )
                msk2 = wpool.tile([1, 128], FP, tag="msk2")
                m2v = msk2[:, :].rearrange("p (j m) -> p m j", j=4)
                nc.vector.tensor_tensor(
                    out=m2v, in0=vrow,
                    in1=gmj[:, :].unsqueeze(2).to_broadcast([1, B, 4]),
                    op=mybir.AluOpType.is_equal,
                )
                tmp2 = wpool.tile([1, 128], FP, tag="tmp2")
                t2v = tmp2[:, :].rearrange("p (j m) -> p m j", j=4)
                nc.vector.tensor_tensor(out=t2v, in0=m2v, in1=irow,
                                        op=mybir.AluOpType.mult)
                nc.vector.tensor_reduce(
                    out=vs32[0:1, 0:B], in_=t2v, axis=mybir.AxisListType.X,
                    op=mybir.AluOpType.add,
                )
                # transpose row -> column, cast to int
                vs32t = wpool.tile([B, B], FP, tag="vs32t")
                nc.vector.transpose(out=vs32t[:, :], in_=vs32[:, :])
                idxi = wpool.tile([B, 1], mybir.dt.int32, tag="idxi")
                nc.vector.tensor_copy(out=idxi[:, :], in_=vs32t[:, 0:1])

                # ---------- gather G'[tok] ----------
                xg = wpool.tile([B, G3], FP, tag="xg")
                nc.gpsimd.indirect_dma_start(
                    out=xg[:, :], out_offset=None,
                    in_=gp_in[:, :],
                    in_offset=IndirectOffsetOnAxis(ap=idxi[:, 0:1], axis=0),
                )

                # ---------- GRU ----------
                rzp = wpool.tile([B, 2 * H], FP, tag="rzp")
                nc.vector.tensor_add(out=rzp[:, :], in0=xg[:, 0:2 * H], in1=ghp[:, 0:2 * H])
                rz = wpool.tile([B, 2 * H], FP, tag="rz")
                nc.scalar.activation(out=rz[:, :], in_=rzp[:, :],
                                     func=mybir.ActivationFunctionType.Sigmoid)
                rh = wpool.tile([B, H], FP, tag="rh")
                nc.vector.tensor_mul(out=rh[:, :], in0=rz[:, 0:H], in1=ghp[:, 2 * H:G3])
                npre = wpool.tile([B, H], FP, tag="npre")
                nc.vector.tensor_add(out=npre[:, :], in0=xg[:, 2 * H:G3], in1=rh[:, :])
                nn_ = wpool.tile([B, H], FP, tag="nn")
                nc.scalar.activation(out=nn_[:, :], in_=npre[:, :],
                                     func=mybir.ActivationFunctionType.Tanh)
                dd = wpool.tile([B, H], FP, tag="dd")
                nc.vector.tensor_sub(out=dd[:, :], in0=ha_sb[:, 0:H], in1=nn_[:, :])
                zd = wpool.tile([B, H], FP, tag="zd")
                nc.vector.tensor_mul(out=zd[:, :], in0=rz[:, H:2 * H], in1=dd[:, :])
                nc.vector.tensor_add(out=ha_sb[:, 0:H], in0=nn_[:, :], in1=zd[:, :])

                # hT update: transpose ha [32, 17] -> [17, 32]
                htt = ptpool.tile([K, B], FP, tag="htt")
                nc.tensor.transpose(out=htt[:, :], in_=ha_sb[:, :],
                                    identity=ident_sb[0:B, 0:B])
                nc.vector.tensor_copy(out=hT_sb[:, :], in_=htt[:, :])

                # ---------- output DMA ----------
                if t % 4 == 3 or t == n_steps - 1:
                    nt = t % 4 + 1
                    t0_ = t - nt + 1
                    for tt in range(nt):
                        dst = AP(out_dram, ((t0_ + tt) % out_steps) * VLOC,
                                 [[1000, 4], [out_steps * VLOC, B], [1, F]])
                        nc.scalar.dma_start(
                            out=dst, in_=slab[:, tt * FPAD:tt * FPAD + F])

    for bi, sem, thr in patches:
        bi.wait_op(sem, thr, "sem-ge")
    nc.finalize()
    return nc


def build_nc_v2(n_steps=T, reps=1, out_steps=None, mm_dtype=FP, debug=False,
                exchange="allgather", bc=True):
    """Lean critical path:
    - scan matmuls optionally float32r (mm_dtype)
    - sender-side j-fold to batch-major candidates; split val/idx remote
      broadcasts so max_index hides under the val exchange
    - receiver combine: 4 small DVE ops
    - transposed GRU state hT [17, 32] (partition 16 = ones), gates in one
      PSUM bank free-sliced at partitions 0:16; embedding rows transposed
      into PSUM via accumulating PE transposes; no final state transpose
    """
    if out_steps is None:
        out_steps = n_steps
    RING = 4
    nc = bacc.Bacc(None, target_bir_lowering=False)

    wt_in = nc.dram_tensor("wt", [K, 4 * FPAD], FP, kind="ExternalInput")
    whh_in = nc.dram_tensor("whh", [K, G3], FP, kind="ExternalInput")
    h0t_in = nc.dram_tensor("h0t", [K, B], FP, kind="ExternalInput")
    off2_in = nc.dram_tensor("off2", [128, 1], FP, kind="ExternalInput")
    ident_in = nc.dram_tensor("ident", [128, 128], FP, kind="ExternalInput")
    gp_in = nc.dram_tensor("gp", [V, G3], FP, kind="ExternalInput")
    out_dram = nc.dram_tensor("out", [B, out_steps, VLOC], FP, kind="ExternalOutput")
    if debug:
        # per step: bm row0 (64), xin cols 0+32 stacked (2*128), rcvV slot (8*32=256 as 128x2?),
        # dump flat: [n_steps, 64 + 256 + 256 + 256 + 32 + 32]
        dbg_dram = nc.dram_tensor("dbg", [n_steps, 64 + 128 * 2 + 256 + 256 + 32 + 32
                                           + 32 * 48 + 17 * 32 + 16 * 224],
                                  FP, kind="ExternalOutput")

    patches = []
    with tile.TileContext(nc) as tc:
        with (
            tc.tile_pool(name="const", bufs=1) as cpool,
            tc.tile_pool(name="state", bufs=1) as spool,
            tc.tile_pool(name="work", bufs=4) as wpool,
            tc.tile_pool(name="stage", bufs=3) as stpool,
            tc.tile_pool(name="psumL", bufs=2, space="PSUM") as plpool,
            tc.tile_pool(name="psumG", bufs=1, space="PSUM") as pgpool,
            tc.tile_pool(name="dram", bufs=4, space="DRAM") as dpool,
        ):
            # constants
            wt_sb = cpool.tile([K, 4 * FPAD], FP, tag="wt")
            nc.gpsimd.dma_start(out=wt_sb[:, :], in_=wt_in[:, :])
            whh_sb = cpool.tile([K, G3], FP, tag="whh")
            nc.gpsimd.dma_start(out=whh_sb[:, :], in_=whh_in[:, :])
            off2_sb = cpool.tile([128, 1], FP, tag="off2")
            nc.gpsimd.dma_start(out=off2_sb[:, :], in_=off2_in[:, :])
            ident_sb = cpool.tile([128, 128], FP, tag="ident")
            nc.gpsimd.dma_start(out=ident_sb[:, :], in_=ident_in[:, :])

            # state: hT_aug [17, 32], partition 16 stays all-ones
            hT_sb = spool.tile([K, B], FP, tag="hT")
            nc.gpsimd.dma_start(out=hT_sb[:, :], in_=h0t_in[:, :])

            # sender-side staging
            bm = spool.tile([B, 64], FP, tag="bm")
            nc.vector.memset(bm[:, :], 0.0)
            xin = spool.tile([128, 64], FP, tag="xin")
            nc.vector.memset(xin[:, :], 0.0)

            for rep in range(reps):
              for t in range(n_steps):
                tg = rep * n_steps + t
                # ---------- scan matmuls ----------
                lps = plpool.tile([128, FPAD], FP, tag="L")
                for j in range(4):
                    for hh in range(2):
                        nc.tensor.matmul(
                            out=lps[32 * j:32 * j + B, hh * 512:(hh + 1) * 512],
                            lhsT=hT_sb[:, :].bitcast(mm_dtype),
                            rhs=wt_sb[:, j * FPAD + hh * 512:j * FPAD + (hh + 1) * 512].bitcast(mm_dtype),
                            start=True, stop=True,
                            tile_position=(0, 32 * j),
                        )
                # gh gates (exact fp32): pt free-sliced [16, 128]:
                #   r: 0:32, z: 32:64, n(gh): 64:96, n(gi): 96:128
                # pt slots (512B-aligned matmul outs):
                # 0 gh_r | 128 gh_z | 256 gh_n | 384 gi_n | 512 gi_r | 640 gi_z
                pt = pgpool.tile([H, 768], FP, tag="pt")
                for g in range(3):
                    nc.tensor.matmul(
                        out=pt[:, 128 * g:128 * g + 32],
                        lhsT=whh_sb[:, 16 * g:16 * g + 16],
                        rhs=hT_sb[:, :],
                        start=True, stop=True, tile_position=(0, 0),
                    )

                # ---------- stage to SBUF for output ----------
                if t % 4 == 0:
                    slab = stpool.tile([128, 4 * FPAD], FP, tag="stg4")
                stg = slab[:, (t % 4) * FPAD:(t % 4 + 1) * FPAD]
                nc.scalar.copy(out=stg, in_=lps[:, :])
                # gh r,z to SBUF (off critical path; avoids 2-PSUM-operand op)
                ghsb = wpool.tile([H, 64], FP, tag="ghsb")
                nc.scalar.copy(out=ghsb[:, 0:32], in_=pt[:, 0:32])
                nc.scalar.copy(out=ghsb[:, 32:64], in_=pt[:, 128:160])

                # ---------- local argmax ----------
                mx8 = wpool.tile([128, 8], FP, tag="mx8")
                nc.vector.max(out=mx8[:, :], in_=lps[:, :])
                mi8 = wpool.tile([128, 8], mybir.dt.uint32, tag="mi8")
                nc.vector.max_index(out=mi8[:, :], in_max=mx8[:, :], in_values=lps[:, :])

                # ---------- sender j-fold (batch-major) ----------
                # tj [1, 256] psum: cols 0:128 = vals(j,m), 128:256 = gidx(j,m)
                tj = pgpool.tile([1, 256], FP, tag="tj")
                tjv = tj[:, 0:128]
                tji = tj[:, 128:256]
                nc.tensor.transpose(out=tjv, in_=mx8[:, 0:1], identity=ident_sb[:, :])
                cwi = wpool.tile([128, 1], FP, tag="cwi")
                nc.vector.tensor_scalar_add(cwi[:, :], mi8[:, 0:1], off2_sb[:, 0:1])
                nc.tensor.transpose(out=tji, in_=cwi[:, :], identity=ident_sb[:, :])

                # bm [32, 64]: row0 block0 = per-batch max val, row0 block1 = idx
                vrow = tjv.rearrange("p (j m) -> p m j", j=4)
                nc.vector.tensor_reduce(
                    out=bm[0:1, 0:B], in_=vrow, axis=mybir.AxisListType.X,
                    op=mybir.AluOpType.max,
                )
                msk = wpool.tile([1, 128], FP, tag="msk")
                mskv = msk[:, :].rearrange("p (j m) -> p m j", j=4)
                nc.vector.tensor_tensor(
                    out=mskv, in0=vrow,
                    in1=bm[0:1, 0:B].unsqueeze(2).to_broadcast([1, B, 4]),
                    op=mybir.AluOpType.is_equal,
                )
                tmpj = wpool.tile([1, 128], FP, tag="tmpj")
                tmpjv = tmpj[:, :].rearrange("p (j m) -> p m j", j=4)
                irow = tji.rearrange("p (j m) -> p m j", j=4)
                nc.vector.tensor_tensor(out=tmpjv, in0=mskv, in1=irow,
                                        op=mybir.AluOpType.mult)
                nc.vector.tensor_reduce(
                    out=bm[0:1, 32:32 + B], in_=tmpjv, axis=mybir.AxisListType.X,
                    op=mybir.AluOpType.add,
                )

                nc.vector.transpose(out=xin[0:B, 0:32], in_=bm[:, 0:32])
                nc.vector.transpose(out=xin[0:B, 32:64], in_=bm[:, 32:64])

                # ---------- exchange: AllGather of [128, 2] batch-major ----------
                # (rows 32:128 are padding; same collective shape as the
                # proven v1 path)
                rcvb = wpool.tile([B, 2 * NCORES], FP, tag="rcvb")
                if exchange == "allgather":
                    cc_in = dpool.tile([128, 2], FP, tag="ccin")
                    cc_out = dpool.tile([128 * NCORES, 2], FP, tag="ccout")
                    nc.scalar.dma_start(
                        out=cc_in[:, :],
                        in_=AP(xin[:, :].tensor, 0, [[64, 128], [32, 2]]))
                    nc.gpsimd.collective_compute(
                        "AllGather",
                        mybir.AluOpType.bypass,
                        ins=[cc_in[:, :].opt()],
                        outs=[cc_out[:, :].opt()],
                        replica_groups=[list(range(NCORES))],
                    )
                    nc.scalar.dma_start(
                        out=rcvb[:, :],
                        in_=AP(cc_out[:, :].tensor, 0,
                               [[2, B], [128 * 2, NCORES], [1, 2]]),
                    )
                else:
                    # timing-only: fake exchange, replicate own candidates
                    nc.vector.tensor_copy(
                        out=rcvb[:, :].rearrange("p (c x) -> p c x", x=2),
                        in_=AP(xin[:, :].tensor, 0,
                               [[64, B], [32, 2]]).unsqueeze(1).to_broadcast(
                                   [B, NCORES, 2]))

                # ---------- receiver combine ----------
                rr = rcvb[:, :].rearrange("p (c x) -> p c x", x=2)
                rv = wpool.tile([B, 1], FP, tag="rv")
                nc.vector.tensor_reduce(
                    out=rv[:, :], in_=rr[:, :, 0],
                    axis=mybir.AxisListType.X, op=mybir.AluOpType.max,
                )
                mskc = wpool.tile([B, NCORES], FP, tag="mskc")
                nc.vector.tensor_scalar(
                    out=mskc[:, :], in0=rr[:, :, 0],
                    scalar1=rv[:, 0:1], scalar2=None,
                    op0=mybir.AluOpType.is_equal,
                )
                junk = wpool.tile([B, NCORES], FP, tag="junk")
                gidx = wpool.tile([B, 1], FP, tag="gidx")
                nc.vector.tensor_tensor(out=junk[:, :], in0=mskc[:, :],
                                        in1=rr[:, :, 1], op=mybir.AluOpType.mult)
                nc.vector.tensor_reduce(
                    out=gidx[:, :], in_=junk[:, :], axis=mybir.AxisListType.X,
                    op=mybir.AluOpType.add,
                )
                idxi = wpool.tile([B, 1], mybir.dt.int32, tag="idxi")
                nc.vector.tensor_copy(out=idxi[:, :], in_=gidx[:, :])
                if debug:
                    dstep = dbg_dram[t:t + 1, :]
                    nc.scalar.dma_start(out=dstep[:, 0:64], in_=bm[0:1, :])
                    nc.scalar.dma_start(out=dstep[:, 64:192],
                                        in_=AP(xin[:, :].tensor, 0, [[64, 128], [1, 1]]))
                    nc.scalar.dma_start(out=dstep[:, 192:320],
                                        in_=AP(xin[:, :].tensor, 32, [[64, 128], [1, 1]]))
                    nc.scalar.dma_start(
                        out=dstep[:, 320:832],
                        in_=AP(rcvb[:, :].tensor, 0, [[16, 32], [1, 16]]))
                    nc.scalar.dma_start(out=dstep[:, 832:864],
                                        in_=AP(rv[:, :].tensor, 0, [[1, 32], [1, 1]]))
                    nc.scalar.dma_start(out=dstep[:, 864:896],
                                        in_=AP(gidx[:, :].tensor, 0, [[1, 32], [1, 1]]))

                # ---------- gather G'[tok] ----------
                xg = wpool.tile([B, G3], FP, tag="xg")
                nc.gpsimd.indirect_dma_start(
                    out=xg[:, :], out_offset=None,
                    in_=gp_in[:, :],
                    in_offset=IndirectOffsetOnAxis(ap=idxi[:, 0:1], axis=0),
                    **(dict(bounds_check=V - 1, oob_is_err=False) if bc else {}),
                )
                # transpose gi gates into separate 512B-aligned psum slots
                nc.tensor.matmul(
                    out=pt[:, 512:544], lhsT=xg[:, 0:16], rhs=ident_sb[0:B, 0:B],
                    start=True, stop=True, is_transpose=True,
                )
                nc.tensor.matmul(
                    out=pt[:, 640:672], lhsT=xg[:, 16:32], rhs=ident_sb[0:B, 0:B],
                    start=True, stop=True, is_transpose=True,
                )
                nc.tensor.matmul(
                    out=pt[:, 384:416], lhsT=xg[:, 32:48], rhs=ident_sb[0:B, 0:B],
                    start=True, stop=True, is_transpose=True,
                )

                # ---------- GRU (all [16, 32] at partitions 0:16) ----------
                rzsum = wpool.tile([H, 64], FP, tag="rzsum")
                nc.vector.tensor_add(out=rzsum[:, 0:32], in0=ghsb[:, 0:32],
                                     in1=pt[:, 512:544])
                nc.vector.tensor_add(out=rzsum[:, 32:64], in0=ghsb[:, 32:64],
                                     in1=pt[:, 640:672])
                rzsig = wpool.tile([H, 64], FP, tag="rzsig")
                nc.scalar.activation(out=rzsig[:, :], in_=rzsum[:, :],
                                     func=mybir.ActivationFunctionType.Sigmoid)
                rh = wpool.tile([H, B], FP, tag="rh")
                nc.vector.tensor_mul(out=rh[:, :], in0=rzsig[:, 0:32],
                                     in1=pt[:, 256:288])
                ns_ = wpool.tile([H, B], FP, tag="ns")
                nc.vector.tensor_add(out=ns_[:, :], in0=rh[:, :], in1=pt[:, 384:416])
                nn_ = wpool.tile([H, B], FP, tag="nn")
                nc.scalar.activation(out=nn_[:, :], in_=ns_[:, :],
                                     func=mybir.ActivationFunctionType.Tanh)
                dd = wpool.tile([H, B], FP, tag="dd")
                nc.vector.tensor_sub(out=dd[:, :], in0=hT_sb[0:H, :], in1=nn_[:, :])
                zd = wpool.tile([H, B], FP, tag="zd")
                nc.vector.tensor_mul(out=zd[:, :], in0=rzsig[:, 32:64], in1=dd[:, :])
                hTw = nc.vector.tensor_add(out=hT_sb[0:H, :], in0=nn_[:, :], in1=zd[:, :])
                if debug:
                    nc.scalar.dma_start(out=dstep[:, 896:896 + 32 * 48],
                                        in_=AP(xg[:, :].tensor, 0, [[48, 32], [1, 48]]))
                    nc.scalar.dma_start(out=dstep[:, 2432:2432 + 17 * 32],
                                        in_=AP(hT_sb[:, :].tensor, 0, [[32, 17], [1, 32]]))
                    ptc = wpool.tile([H, 224], FP, tag="ptdbg")
                    nc.vector.memset(ptc[:, :], 0.0)
                    for si, so in ((0, 0), (128, 32), (256, 64), (384, 96),
                                   (512, 128), (640, 160)):
                        nc.vector.tensor_copy(out=ptc[:, so:so + 32],
                                              in_=pt[:, si:si + 32])
                    nc.scalar.dma_start(out=dstep[:, 2976:2976 + 16 * 224],
                                        in_=AP(ptc[:, :].tensor, 0, [[224, 16], [1, 224]]))

                # ---------- output DMA ----------
                if t % 4 == 3 or t == n_steps - 1:
                    nt = t % 4 + 1
                    t0_ = t - nt + 1
                    for tt in range(nt):
                        dst = AP(out_dram, ((t0_ + tt) % out_steps) * VLOC,
                                 [[1000, 4], [out_steps * VLOC, B], [1, F]])
                        nc.scalar.dma_start(
                            out=dst, in_=slab[:, tt * FPAD:tt * FPAD + F])

    for bi, sem, thr in patches:
        bi.wait_op(sem, thr, "sem-ge")
    nc.finalize()
    return nc


def build_nc_v6(n_steps=T, reps=1, out_steps=None, exchange="allgather",
                gather="indirect", split_argmax=True, exch_eng="sp",
                payload=32, pe_warm=0, mm_dtype=FP, stage=True,
                out_fmt="btv", odt=FP, out_eng="sc"):
    """Lean single-chain cycle, v2 skeleton plus:
    - exchange DMAs on the SP (sync) engine so they never queue behind the
      Activation-engine stage copy (exch_eng: sp|sc)
    - [32, 2] collective payload instead of [128, 2] (payload: 32|128)
    - split argmax: max/max_index on free halves, first half overlapped with
      the scan's second half; merged with 4 tiny DVE ops
    - optional PE-warming dummy transposes during the exchange window
      (pe_warm = number of dummy [128,128] transposes)
    - gather="fixed" ablation: constant row-0 gather (breaks correctness,
      removes exchange->gather dependency for timing)
    - exchange="fake" ablation: replicate own candidates, no collective
    """
    if out_steps is None:
        out_steps = n_steps
    nc = bacc.Bacc(None, target_bir_lowering=False)

    wt_in = nc.dram_tensor("wt", [K, 4 * FPAD], FP, kind="ExternalInput")
    whh_in = nc.dram_tensor("whh", [K, G3], FP, kind="ExternalInput")
    h0t_in = nc.dram_tensor("h0t", [K, B], FP, kind="ExternalInput")
    off2_in = nc.dram_tensor("off2", [128, 1], FP, kind="ExternalInput")
    ident_in = nc.dram_tensor("ident", [128, 128], FP, kind="ExternalInput")
    gp_in = nc.dram_tensor("gp", [V, G3], FP, kind="ExternalInput")
    if out_fmt == "tjbf":
        # partition-major: row t = [4*B partitions x F] contiguous; host
        # reassembles [T, 4, B, F] -> [B, T, 4F]. One clean DMA per step.
        out_dram = nc.dram_tensor("out", [out_steps, 128 * F], odt,
                                  kind="ExternalOutput")
    else:
        out_dram = nc.dram_tensor("out", [B, out_steps, VLOC], FP,
                                  kind="ExternalOutput")

    with tile.TileContext(nc) as tc:
        with (
            tc.tile_pool(name="const", bufs=1) as cpool,
            tc.tile_pool(name="state", bufs=1) as spool,
            tc.tile_pool(name="work", bufs=4) as wpool,
            tc.tile_pool(name="stage", bufs=3) as stpool,
            tc.tile_pool(name="psumL", bufs=2, space="PSUM") as plpool,
            tc.tile_pool(name="psumG", bufs=1, space="PSUM") as pgpool,
            tc.tile_pool(name="psumT", bufs=1, space="PSUM") as ptpool,
            tc.tile_pool(name="dram", bufs=4, space="DRAM") as dpool,
        ):
            xeng = nc.sync if exch_eng == "sp" else nc.scalar
            oeng = {"sc": nc.scalar, "gp": nc.gpsimd, "sp": nc.sync}[out_eng]

            wt_sb = cpool.tile([K, 4 * FPAD], FP, tag="wt")
            nc.gpsimd.dma_start(out=wt_sb[:, :], in_=wt_in[:, :])
            whh_sb = cpool.tile([K, G3], FP, tag="whh")
            nc.gpsimd.dma_start(out=whh_sb[:, :], in_=whh_in[:, :])
            off2_sb = cpool.tile([128, 1], FP, tag="off2")
            nc.gpsimd.dma_start(out=off2_sb[:, :], in_=off2_in[:, :])
            ident_sb = cpool.tile([128, 128], FP, tag="ident")
            nc.gpsimd.dma_start(out=ident_sb[:, :], in_=ident_in[:, :])

            hT_sb = spool.tile([K, B], FP, tag="hT")
            nc.gpsimd.dma_start(out=hT_sb[:, :], in_=h0t_in[:, :])

            bm = spool.tile([B, 64], FP, tag="bm")
            nc.vector.memset(bm[:, :], 0.0)
            xin = spool.tile([128, 64], FP, tag="xin")
            nc.vector.memset(xin[:, :], 0.0)
            c512_sb = cpool.tile([128, 1], FP, tag="c512")
            nc.vector.memset(c512_sb[:, :], 512.0)

            for rep in range(reps):
              for t in range(n_steps):
                # ---------- scan matmuls: hh-major so argmax can split ----------
                lps = plpool.tile([128, FPAD], FP, tag="L")
                for hh in range(2):
                    for j in range(4):
                        nc.tensor.matmul(
                            out=lps[32 * j:32 * j + B, hh * 512:(hh + 1) * 512],
                            lhsT=hT_sb[:, :].bitcast(mm_dtype),
                            rhs=wt_sb[:, j * FPAD + hh * 512:j * FPAD + (hh + 1) * 512].bitcast(mm_dtype),
                            start=True, stop=True,
                            tile_position=(0, 32 * j),
                        )
                # gh gates from current h (pre-exchange, exact fp32)
                pt = pgpool.tile([H, 768], FP, tag="pt")
                for g in range(3):
                    nc.tensor.matmul(
                        out=pt[:, 128 * g:128 * g + 32],
                        lhsT=whh_sb[:, 16 * g:16 * g + 16],
                        rhs=hT_sb[:, :],
                        start=True, stop=True, tile_position=(0, 0),
                    )

                # ---------- stage to SBUF for output (off critical path) -------
                if stage:
                    if t % 4 == 0:
                        slab = stpool.tile([128, 4 * FPAD], odt, tag="stg4")
                    stg = slab[:, (t % 4) * FPAD:(t % 4 + 1) * FPAD]
                    if stage == "vec":
                        nc.vector.tensor_copy(out=stg, in_=lps[:, :])
                    elif stage == "gp":
                        nc.gpsimd.tensor_copy(out=stg, in_=lps[:, :])
                    elif stage != "dmaonly":
                        nc.scalar.copy(out=stg, in_=lps[:, :])
                ghsb = wpool.tile([H, 64], FP, tag="ghsb")
                nc.scalar.copy(out=ghsb[:, 0:32], in_=pt[:, 0:32])
                nc.scalar.copy(out=ghsb[:, 32:64], in_=pt[:, 128:160])

                # ---------- local argmax ----------
                if split_argmax:
                    mxh = wpool.tile([128, 16], FP, tag="mxh")
                    mih = wpool.tile([128, 16], mybir.dt.uint32, tag="mih")
                    nc.vector.max(out=mxh[:, 0:8], in_=lps[:, 0:512])
                    nc.vector.max_index(out=mih[:, 0:8], in_max=mxh[:, 0:8],
                                        in_values=lps[:, 0:512])
                    nc.vector.max(out=mxh[:, 8:16], in_=lps[:, 512:1024])
                    nc.vector.max_index(out=mih[:, 8:16], in_max=mxh[:, 8:16],
                                        in_values=lps[:, 512:1024])
                    # merge halves: win = B>A ? (valB, idxB+512) : (valA, idxA)
                    mif = wpool.tile([128, 2], FP, tag="mif")
                    nc.vector.tensor_copy(out=mif[:, 0:2], in_=mih[:, 0:16:8])
                    cmp_ = wpool.tile([128, 1], FP, tag="cmp")
                    nc.vector.tensor_tensor(out=cmp_[:, :], in0=mxh[:, 8:9],
                                            in1=mxh[:, 0:1],
                                            op=mybir.AluOpType.is_gt)
                    mx1 = wpool.tile([128, 1], FP, tag="mx1")
                    nc.vector.tensor_reduce(out=mx1[:, :], in_=mxh[:, 0:16:8],
                                            axis=mybir.AxisListType.X,
                                            op=mybir.AluOpType.max)
                    # idx = idxA + cmp * (idxB + 512 - idxA)
                    dif = wpool.tile([128, 1], FP, tag="dif")
                    nc.vector.tensor_sub(out=dif[:, :], in0=mif[:, 1:2], in1=mif[:, 0:1])
                    nc.vector.tensor_scalar_add(dif[:, :], dif[:, :], c512_sb[:, 0:1])
                    md = wpool.tile([128, 1], FP, tag="md")
                    nc.vector.tensor_mul(out=md[:, :], in0=cmp_[:, :], in1=dif[:, :])
                    mi1 = wpool.tile([128, 1], FP, tag="mi1")
                    nc.vector.tensor_add(out=mi1[:, :], in0=mif[:, 0:1], in1=md[:, :])
                    cwi = wpool.tile([128, 1], FP, tag="cwi")
                    nc.vector.tensor_scalar_add(cwi[:, :], mi1[:, 0:1], off2_sb[:, 0:1])
                    mxsrc = mx1
                else:
                    mx8 = wpool.tile([128, 8], FP, tag="mx8")
                    nc.vector.max(out=mx8[:, :], in_=lps[:, :])
                    mi8 = wpool.tile([128, 8], mybir.dt.uint32, tag="mi8")
                    nc.vector.max_index(out=mi8[:, :], in_max=mx8[:, :], in_values=lps[:, :])
                    cwi = wpool.tile([128, 1], FP, tag="cwi")
                    nc.vector.tensor_scalar_add(cwi[:, :], mi8[:, 0:1], off2_sb[:, 0:1])
                    mxsrc = mx8

                # ---------- sender j-fold (batch-major) ----------
                tj = ptpool.tile([1, 256], FP, tag="tj")
                tjv = tj[:, 0:128]
                tji = tj[:, 128:256]
                nc.tensor.transpose(out=tjv, in_=mxsrc[:, 0:1], identity=ident_sb[:, :])
                nc.tensor.transpose(out=tji, in_=cwi[:, :], identity=ident_sb[:, :])

                vrow = tjv.rearrange("p (j m) -> p m j", j=4)
                nc.vector.tensor_reduce(
                    out=bm[0:1, 0:B], in_=vrow, axis=mybir.AxisListType.X,
                    op=mybir.AluOpType.max,
                )
                msk = wpool.tile([1, 128], FP, tag="msk")
                mskv = msk[:, :].rearrange("p (j m) -> p m j", j=4)
                nc.vector.tensor_tensor(
                    out=mskv, in0=vrow,
                    in1=bm[0:1, 0:B].unsqueeze(2).to_broadcast([1, B, 4]),
                    op=mybir.AluOpType.is_equal,
                )
                tmpj = wpool.tile([1, 128], FP, tag="tmpj")
                tmpjv = tmpj[:, :].rearrange("p (j m) -> p m j", j=4)
                irow = tji.rearrange("p (j m) -> p m j", j=4)
                nc.vector.tensor_tensor(out=tmpjv, in0=mskv, in1=irow,
                                        op=mybir.AluOpType.mult)
                nc.vector.tensor_reduce(
                    out=bm[0:1, 32:32 + B], in_=tmpjv,
                    axis=mybir.AxisListType.X, op=mybir.AluOpType.add,
                )
                nc.vector.transpose(out=xin[0:B, 0:32], in_=bm[:, 0:32])
                nc.vector.transpose(out=xin[0:B, 32:64], in_=bm[:, 32:64])

                # ---------- exchange ----------
                PN = payload
                rcvb = wpool.tile([B, 2 * NCORES], FP, tag="rcvb")
                if exchange == "allgather":
                    cc_in = dpool.tile([PN, 2], FP, tag="ccin")
                    cc_out = dpool.tile([PN * NCORES, 2], FP, tag="ccout")
                    xeng.dma_start(
                        out=cc_in[:, :],
                        in_=AP(xin[:, :].tensor, 0, [[64, PN], [32, 2]]))
                    nc.gpsimd.collective_compute(
                        "AllGather",
                        mybir.AluOpType.bypass,
                        ins=[cc_in[:, :].opt()],
                        outs=[cc_out[:, :].opt()],
                        replica_groups=[list(range(NCORES))],
                    )
                    xeng.dma_start(
                        out=rcvb[:, :],
                        in_=AP(cc_out[:, :].tensor, 0,
                               [[2, B], [PN * 2, NCORES], [1, 2]]),
                    )
                else:
                    nc.vector.tensor_copy(
                        out=rcvb[:, :].rearrange("p (c x) -> p c x", x=2),
                        in_=AP(xin[:, :].tensor, 0,
                               [[64, B], [32, 2]]).unsqueeze(1).to_broadcast(
                                   [B, NCORES, 2]))

                # PE warm dummies during the exchange window
                for w in range(pe_warm):
                    dwp = ptpool.tile([128, 128], FP, tag="dw")
                    nc.tensor.transpose(out=dwp[:, :], in_=ident_sb[:, :],
                                        identity=ident_sb[:, :])

                # ---------- receiver combine ----------
                rr = rcvb[:, :].rearrange("p (c x) -> p c x", x=2)
                rv = wpool.tile([B, 1], FP, tag="rv")
                nc.vector.tensor_reduce(
                    out=rv[:, :], in_=rr[:, :, 0],
                    axis=mybir.AxisListType.X, op=mybir.AluOpType.max,
                )
                mskc = wpool.tile([B, NCORES], FP, tag="mskc")
                nc.vector.tensor_scalar(
                    out=mskc[:, :], in0=rr[:, :, 0],
                    scalar1=rv[:, 0:1], scalar2=None,
                    op0=mybir.AluOpType.is_equal,
                )
                junk = wpool.tile([B, NCORES], FP, tag="junk")
                gidx = wpool.tile([B, 1], FP, tag="gidx")
                nc.vector.tensor_tensor(out=junk[:, :], in0=mskc[:, :],
                                        in1=rr[:, :, 1], op=mybir.AluOpType.mult)
                nc.vector.tensor_reduce(
                    out=gidx[:, :], in_=junk[:, :], axis=mybir.AxisListType.X,
                    op=mybir.AluOpType.add,
                )
                idxi = wpool.tile([B, 1], mybir.dt.int32, tag="idxi")
                nc.vector.tensor_copy(out=idxi[:, :], in_=gidx[:, :])

                # ---------- gather G'[tok] ----------
                xg = wpool.tile([B, G3], FP, tag="xg")
                if gather == "indirect":
                    nc.gpsimd.indirect_dma_start(
                        out=xg[:, :], out_offset=None,
                        in_=gp_in[:, :],
                        in_offset=IndirectOffsetOnAxis(ap=idxi[:, 0:1], axis=0),
                        bounds_check=V - 1, oob_is_err=False,
                    )
                else:
                    nc.gpsimd.dma_start(out=xg[:, :], in_=gp_in[0:B, :])
                # transpose gi gates into 512B-aligned psum slots
                nc.tensor.matmul(
                    out=pt[:, 512:544], lhsT=xg[:, 0:16], rhs=ident_sb[0:B, 0:B],
                    start=True, stop=True, is_transpose=True,
                )
                nc.tensor.matmul(
                    out=pt[:, 640:672], lhsT=xg[:, 16:32], rhs=ident_sb[0:B, 0:B],
                    start=True, stop=True, is_transpose=True,
                )
                nc.tensor.matmul(
                    out=pt[:, 384:416], lhsT=xg[:, 32:48], rhs=ident_sb[0:B, 0:B],
                    start=True, stop=True, is_transpose=True,
                )

                # ---------- GRU ----------
                rzsum = wpool.tile([H, 64], FP, tag="rzsum")
                nc.vector.tensor_add(out=rzsum[:, 0:32], in0=ghsb[:, 0:32],
                                     in1=pt[:, 512:544])
                nc.vector.tensor_add(out=rzsum[:, 32:64], in0=ghsb[:, 32:64],
                                     in1=pt[:, 640:672])
                rzsig = wpool.tile([H, 64], FP, tag="rzsig")
                nc.scalar.activation(out=rzsig[:, :], in_=rzsum[:, :],
                                     func=mybir.ActivationFunctionType.Sigmoid)
                rh = wpool.tile([H, B], FP, tag="rh")
                nc.vector.tensor_mul(out=rh[:, :], in0=rzsig[:, 0:32],
                                     in1=pt[:, 256:288])
                ns_ = wpool.tile([H, B], FP, tag="ns")
                nc.vector.tensor_add(out=ns_[:, :], in0=rh[:, :], in1=pt[:, 384:416])
                nn_ = wpool.tile([H, B], FP, tag="nn")
                nc.scalar.activation(out=nn_[:, :], in_=ns_[:, :],
                                     func=mybir.ActivationFunctionType.Tanh)
                dd = wpool.tile([H, B], FP, tag="dd")
                nc.vector.tensor_sub(out=dd[:, :], in0=hT_sb[0:H, :], in1=nn_[:, :])
                zd = wpool.tile([H, B], FP, tag="zd")
                nc.vector.tensor_mul(out=zd[:, :], in0=rzsig[:, 32:64], in1=dd[:, :])
                nc.vector.tensor_add(out=hT_sb[0:H, :], in0=nn_[:, :], in1=zd[:, :])

                # ---------- output DMA ----------
                if stage and stage != "copyonly" and (t % 4 == 3 or t == n_steps - 1):
                    nt = t % 4 + 1
                    t0_ = t - nt + 1
                    for tt in range(nt):
                        if out_fmt == "tjbf":
                            dst = AP(out_dram,
                                     ((t0_ + tt) % out_steps) * 128 * F,
                                     [[F, 128], [1, F]])
                        else:
                            dst = AP(out_dram, ((t0_ + tt) % out_steps) * VLOC,
                                     [[1000, 4], [out_steps * VLOC, B], [1, F]])
                        oeng.dma_start(
                            out=dst, in_=slab[:, tt * FPAD:tt * FPAD + F])

    nc.finalize()
    return nc


def host_prep(inputs, n_steps=T):
    """Build per-core input maps from the full problem inputs."""
    emb = np.asarray(inputs["embedding"], np.float32)
    W_ih = np.asarray(inputs["W_ih"], np.float32)
    W_hh = np.asarray(inputs["W_hh"], np.float32)
    b_ih = np.asarray(inputs["b_ih"], np.float32)
    b_hh = np.asarray(inputs["b_hh"], np.float32)
    W_out = np.asarray(inputs["W_out"], np.float32)
    b_out = np.asarray(inputs["b_out"], np.float32)
    h0 = np.asarray(inputs["encoder_hidden"], np.float32)[0]  # [B, H]

    # G' = emb @ W_ih.T + b_ih  [V, 48]
    gp = (emb @ W_ih.T + b_ih).astype(np.float32)
    # W_aug [V, 17]
    w_aug = np.concatenate([W_out, b_out[:, None]], axis=1).astype(np.float32)
    # whh_aug.T [17, 48]
    whh = np.concatenate([W_hh.T, b_hh[None, :]], axis=0).astype(np.float32)

    # The kernel's iteration t computes logits_t from its current state, so the
    # initial state must be h1 = GRU(emb[SOS=0], h0), computed here in fp32.
    x0 = np.broadcast_to(emb[0], (B, E))
    gi = (x0 @ W_ih.T + b_ih).astype(np.float32)
    gh = (h0 @ W_hh.T + b_hh).astype(np.float32)
    i_r, i_z, i_n = gi[:, :H], gi[:, H:2 * H], gi[:, 2 * H:]
    h_r, h_z, h_n = gh[:, :H], gh[:, H:2 * H], gh[:, 2 * H:]
    r = (1.0 / (1.0 + np.exp(-(i_r + h_r), dtype=np.float32))).astype(np.float32)
    z = (1.0 / (1.0 + np.exp(-(i_z + h_z), dtype=np.float32))).astype(np.float32)
    n = np.tanh(i_n + r * h_n, dtype=np.float32).astype(np.float32)
    h1 = ((1.0 - z) * n + z * h0).astype(np.float32)

    h0a = np.concatenate([h1, np.ones((B, 1), np.float32)], axis=1)  # [32, 17]
    h0t = h0a.T.copy()                                               # [17, 32]

    ident = np.eye(128, dtype=np.float32)

    in_maps = []
    for c in range(NCORES):
        wt = np.zeros((K, 4 * FPAD), np.float32)
        for j in range(4):
            blk = np.zeros((K, FPAD), np.float32)
            blk[K - 1, :] = -1.0e30          # pad slots: bias -inf
            v0 = c * VLOC + j * 1000
            blk[:, 0:F] = w_aug[v0:v0 + F, :].T
            wt[:, j * FPAD:(j + 1) * FPAD] = blk
        off2 = np.zeros((128, 1), np.float32)
        for j in range(4):
            off2[32 * j:32 * j + 32, 0] = c * VLOC + j * 1000
        in_maps.append({
            "wt": wt, "whh": whh, "h0t": h0t, "h0a": h0a,
            "off2": off2, "ident": ident, "gp": gp,
        })
    return in_maps


def assemble_output(results, n_steps=T):
    """Concatenate per-core [B, T, VLOC] stripes into [B, T, V]."""
    return np.concatenate([r["out"] for r in results], axis=2)


_NC_CACHE = {}

# Best validated configuration for the kernel() entrypoint:
#   "v1":     single-chain, allgather exchange   (~35 us/step)
#   "v6":     lean single-chain: SP-engine exchange DMAs, [32,2] payload,
#             split argmax, contiguous [T, 128*F] output layout
BEST = "v6"


def _build_best(n_steps=T, reps=1, out_steps=None):
    if BEST == "v1":
        return build_nc(n_steps=n_steps, exchange="allgather", reps=reps,
                        out_steps=out_steps)
    if BEST == "v6":
        return build_nc_v6(n_steps=n_steps, reps=reps, out_steps=out_steps,
                           out_fmt="tjbf")
    if BEST == "v4n8":
        return build_nc_v4(n_steps=n_steps, nch=8, reps=reps,
                           out_steps=out_steps)
    if BEST == "v4n8bf":
        return build_nc_v4(n_steps=n_steps, nch=8, reps=reps,
                           out_steps=out_steps, bf16pair=True)
    raise ValueError(BEST)


def _prep_best(inputs):
    if BEST in ("v1", "v6"):
        maps = host_prep(inputs)
        if BEST == "v6":
            maps = [{k: v for k, v in m.items() if k != "h0a"} for m in maps]
        return maps
    return host_prep_v3(inputs, nch=8, bf16pair=BEST.endswith("bf"))


def assemble_output_v6(results, n_steps=T):
    """[T, 128*F] per core -> [B, T, V]: rows are (j, b) partition-major."""
    parts = []
    for r in results:
        arr = np.asarray(r["out"]).astype(np.float32)
        arr = arr.reshape(n_steps, 4, B, F).transpose(2, 0, 1, 3)
        parts.append(arr.reshape(B, n_steps, 4 * F))
    return np.concatenate(parts, axis=2)


def kernel(**inputs):
    """Full-input entrypoint: shard across 8 NeuronCores, run the Bass kernel,
    return the full (32, 100, 32000) float32 logits tensor."""
    from concourse.bass_utils import run_bass_kernel_spmd

    key = (BEST, T)
    if key not in _NC_CACHE:
        _NC_CACHE[key] = _build_best(n_steps=T)
    nc = _NC_CACHE[key]
    in_maps = _prep_best(inputs)
    res = run_bass_kernel_spmd(nc, in_maps, core_ids=list(range(NCORES)))
    if BEST == "v6":
        return assemble_output_v6(res.results)
    return assemble_output(res.results)


# ---------------------------------------------------------------------------
# v3: N-way batch-interleaved chains; each chain's collective exchange is in
# flight while the other chains compute, hiding the ~28us collective latency.
# Slot s advances chain c = s % N by one step. Per chain-step shapes:
#   batches BCH = 32/N; vocab groups G = 4N (4 PE col positions x N k-offsets
#   packed via zero-padded accumulating matmuls); PSUM scan tile [128, FG].
# ---------------------------------------------------------------------------

def build_nc_v3(n_steps=T, nch=4, reps=1, out_steps=None, bf16pair=False):
    if out_steps is None:
        out_steps = n_steps
    BCH = B // nch              # batches per chain
    PPOS = 32 // BCH            # k-offsets per column position (= nch)
    G = 4 * PPOS                # vocab groups
    VG = VLOC // G              # valid vocab per group
    FG = 1 << (VG - 1).bit_length()  # padded free size per group
    PW = 32 * PPOS              # padded stationary width per chain
    KS = 32 + BCH               # stride of h-position across k blocks

    nc = bacc.Bacc(None, target_bir_lowering=False)

    BF16 = mybir.dt.bfloat16
    if bf16pair:
        wth_in = nc.dram_tensor("wt3h", [K, G * FG], BF16, kind="ExternalInput")
        wtl_in = nc.dram_tensor("wt3l", [K, G * FG], BF16, kind="ExternalInput")
        hph_in = nc.dram_tensor("hp0h", [K, nch * PW], BF16, kind="ExternalInput")
        hpl_in = nc.dram_tensor("hp0l", [K, nch * PW], BF16, kind="ExternalInput")
    else:
        wt_in = nc.dram_tensor("wt3", [K, G * FG], FP, kind="ExternalInput")
    whh_in = nc.dram_tensor("whh", [K, G3], FP, kind="ExternalInput")
    hp0_in = nc.dram_tensor("hp0", [K, nch * PW], FP, kind="ExternalInput")
    off3_in = nc.dram_tensor("off3", [128, 1], FP, kind="ExternalInput")
    ident_in = nc.dram_tensor("ident", [128, 128], FP, kind="ExternalInput")
    gp_in = nc.dram_tensor("gp", [V, G3], FP, kind="ExternalInput")
    out_dram = nc.dram_tensor("out", [B, out_steps, VLOC], FP, kind="ExternalOutput")

    with tile.TileContext(nc) as tc:
        with (
            tc.tile_pool(name="const", bufs=1) as cpool,
            tc.tile_pool(name="state", bufs=1) as spool,
            tc.tile_pool(name="work", bufs=4) as wpool,
            tc.tile_pool(name="stage", bufs=4) as stpool,
            tc.tile_pool(name="psumL", bufs=2, space="PSUM") as plpool,
            tc.tile_pool(name="psumG", bufs=2, space="PSUM") as pgpool,
            tc.tile_pool(name="psumT", bufs=2, space="PSUM") as ptpool,
            tc.tile_pool(name="dram", bufs=2 * nch + 2, space="DRAM") as dpool,
        ):
            if bf16pair:
                wth_sb = cpool.tile([K, G * FG], BF16, tag="wth")
                nc.gpsimd.dma_start(out=wth_sb[:, :], in_=wth_in[:, :])
                wtl_sb = cpool.tile([K, G * FG], BF16, tag="wtl")
                nc.gpsimd.dma_start(out=wtl_sb[:, :], in_=wtl_in[:, :])
            else:
                wt_sb = cpool.tile([K, G * FG], FP, tag="wt")
                nc.gpsimd.dma_start(out=wt_sb[:, :], in_=wt_in[:, :])
            whh_sb = cpool.tile([K, G3], FP, tag="whh")
            nc.gpsimd.dma_start(out=whh_sb[:, :], in_=whh_in[:, :])
            off3_sb = cpool.tile([128, 1], FP, tag="off3")
            nc.gpsimd.dma_start(out=off3_sb[:, :], in_=off3_in[:, :])
            ident_sb = cpool.tile([128, 128], FP, tag="ident")
            nc.gpsimd.dma_start(out=ident_sb[:, :], in_=ident_in[:, :])

            # per-chain zero-padded stationary state [17, PW]
            hp = []
            for c in range(nch):
                t_ = spool.tile([K, PW], FP, tag=f"hp{c}")
                nc.gpsimd.dma_start(out=t_[:, :], in_=hp0_in[:, c * PW:(c + 1) * PW])
                hp.append(t_)
            if bf16pair:
                hph, hpl = [], []
                for c in range(nch):
                    th = spool.tile([K, PW], BF16, tag=f"hph{c}")
                    nc.gpsimd.dma_start(out=th[:, :], in_=hph_in[:, c * PW:(c + 1) * PW])
                    hph.append(th)
                    tl = spool.tile([K, PW], BF16, tag=f"hpl{c}")
                    nc.gpsimd.dma_start(out=tl[:, :], in_=hpl_in[:, c * PW:(c + 1) * PW])
                    hpl.append(tl)

            bm = spool.tile([B, 64], FP, tag="bm")
            nc.vector.memset(bm[:, :], 0.0)
            xin = spool.tile([128, 64], FP, tag="xin")
            nc.vector.memset(xin[:, :], 0.0)

            pend = {}  # chain -> cc_out tile of its in-flight exchange

            for rep in range(reps):
              for s in range(nch * n_steps):
                c = s % nch
                t = s // nch
                hpc = hp[c]

                if t >= 1:
                    # ---- gh matmuls from h_{t-1} (needed by GRU below) ----
                    pt = pgpool.tile([H, 768], FP, tag="pt")
                    for g in range(3):
                        nc.tensor.matmul(
                            out=pt[:, 128 * g:128 * g + BCH],
                            lhsT=whh_sb[:, 16 * g:16 * g + 16],
                            rhs=hpc[:, 0:BCH],
                            start=True, stop=True, tile_position=(0, 0),
                        )
                    ghsb = wpool.tile([H, 2 * BCH], FP, tag="ghsb")
                    nc.scalar.copy(
                        out=ghsb[:, :].rearrange("p (x f) -> p x f", x=2),
                        in_=pt[:, 0:256].rearrange(
                            "p (x f) -> p x f", x=2)[:, :, 0:BCH])

                    # ---- consume pending exchange of step t-1 ----
                    cc_out = pend.pop(c)
                    rcvb = wpool.tile([BCH, 2 * NCORES], FP, tag="rcvb")
                    nc.scalar.dma_start(
                        out=rcvb[:, :],
                        in_=AP(cc_out[:, :].tensor, 0,
                               [[2, BCH], [128 * 2, NCORES], [1, 2]]),
                    )
                    rr = rcvb[:, :].rearrange("p (c x) -> p c x", x=2)
                    rv = wpool.tile([BCH, 1], FP, tag="rv")
                    nc.vector.tensor_reduce(
                        out=rv[:, :], in_=rr[:, :, 0],
                        axis=mybir.AxisListType.X, op=mybir.AluOpType.max,
                    )
                    mskc = wpool.tile([BCH, NCORES], FP, tag="mskc")
                    nc.vector.tensor_scalar(
                        out=mskc[:, :], in0=rr[:, :, 0],
                        scalar1=rv[:, 0:1], scalar2=None,
                        op0=mybir.AluOpType.is_equal,
                    )
                    junk = wpool.tile([BCH, NCORES], FP, tag="junk")
                    nc.vector.tensor_tensor(out=junk[:, :], in0=mskc[:, :],
                                            in1=rr[:, :, 1],
                                            op=mybir.AluOpType.mult)
                    gidx = wpool.tile([BCH, 1], FP, tag="gidx")
                    nc.vector.tensor_reduce(
                        out=gidx[:, :], in_=junk[:, :],
                        axis=mybir.AxisListType.X, op=mybir.AluOpType.add,
                    )
                    idxi = wpool.tile([BCH, 1], mybir.dt.int32, tag="idxi")
                    nc.vector.tensor_copy(out=idxi[:, :], in_=gidx[:, :])

                    xg = wpool.tile([BCH, G3], FP, tag="xg")
                    nc.gpsimd.indirect_dma_start(
                        out=xg[:, :], out_offset=None,
                        in_=gp_in[:, :],
                        in_offset=IndirectOffsetOnAxis(ap=idxi[:, 0:1], axis=0),
                    )
                    # gi gate transposes into 512B-aligned psum slots
                    nc.tensor.matmul(
                        out=pt[:, 512:512 + BCH], lhsT=xg[:, 0:16],
                        rhs=ident_sb[0:BCH, 0:BCH],
                        start=True, stop=True, is_transpose=True,
                    )
                    nc.tensor.matmul(
                        out=pt[:, 640:640 + BCH], lhsT=xg[:, 16:32],
                        rhs=ident_sb[0:BCH, 0:BCH],
                        start=True, stop=True, is_transpose=True,
                    )
                    nc.tensor.matmul(
                        out=pt[:, 384:384 + BCH], lhsT=xg[:, 32:48],
                        rhs=ident_sb[0:BCH, 0:BCH],
                        start=True, stop=True, is_transpose=True,
                    )

                    # ---- GRU ----
                    rzsum = wpool.tile([H, 2 * BCH], FP, tag="rzsum")
                    nc.vector.tensor_add(
                        out=rzsum[:, :].rearrange("p (x f) -> p x f", x=2),
                        in0=ghsb[:, :].rearrange("p (x f) -> p x f", x=2),
                        in1=pt[:, 512:768].rearrange(
                            "p (x f) -> p x f", x=2)[:, :, 0:BCH])
                    rzsig = wpool.tile([H, 2 * BCH], FP, tag="rzsig")
                    nc.scalar.activation(out=rzsig[:, :], in_=rzsum[:, :],
                                         func=mybir.ActivationFunctionType.Sigmoid)
                    rh = wpool.tile([H, BCH], FP, tag="rh")
                    nc.vector.tensor_mul(out=rh[:, :], in0=rzsig[:, 0:BCH],
                                         in1=pt[:, 256:256 + BCH])
                    ns_ = wpool.tile([H, BCH], FP, tag="ns")
                    nc.vector.tensor_add(out=ns_[:, :], in0=rh[:, :],
                                         in1=pt[:, 384:384 + BCH])
                    nn_ = wpool.tile([H, BCH], FP, tag="nn")
                    nc.scalar.activation(out=nn_[:, :], in_=ns_[:, :],
                                         func=mybir.ActivationFunctionType.Tanh)
                    dd = wpool.tile([H, BCH], FP, tag="dd")
                    nc.vector.tensor_sub(out=dd[:, :], in0=hpc[0:H, 0:BCH],
                                         in1=nn_[:, :])
                    zd = wpool.tile([H, BCH], FP, tag="zd")
                    nc.vector.tensor_mul(out=zd[:, :], in0=rzsig[:, BCH:2 * BCH],
                                         in1=dd[:, :])
                    hn = wpool.tile([H, BCH], FP, tag="hn")
                    nc.vector.tensor_add(out=hn[:, :], in0=nn_[:, :], in1=zd[:, :])
                    # scatter h_t into all k-offset blocks of hpc
                    nc.vector.tensor_copy(
                        out=AP(hpc[:, :].tensor, 0,
                               [[PW, H], [KS, PPOS], [1, BCH]]),
                        in_=hn[:, :].unsqueeze(1).to_broadcast([H, PPOS, BCH]),
                    )
                    if bf16pair:
                        nc.vector.tensor_copy(
                            out=AP(hph[c][:, :].tensor, 0,
                                   [[PW, H], [KS, PPOS], [1, BCH]]),
                            in_=hn[:, :].unsqueeze(1).to_broadcast([H, PPOS, BCH]),
                        )
                        hlo = wpool.tile([H, BCH], FP, tag="hlo")
                        nc.vector.tensor_sub(out=hlo[:, :], in0=hn[:, :],
                                             in1=hph[c][0:H, 0:BCH])
                        nc.vector.tensor_copy(
                            out=AP(hpl[c][:, :].tensor, 0,
                                   [[PW, H], [KS, PPOS], [1, BCH]]),
                            in_=hlo[:, :].unsqueeze(1).to_broadcast([H, PPOS, BCH]),
                        )

                # ---- scan: G accumulating matmuls ----
                lps = plpool.tile([128, FG], FP, tag="L")
                for j in range(4):
                    for k in range(PPOS):
                        g = PPOS * j + k
                        if bf16pair:
                            terms = [(hph[c], wth_sb), (hpl[c], wth_sb),
                                     (hph[c], wtl_sb)]
                            for ti, (hs, ws) in enumerate(terms):
                                nc.tensor.matmul(
                                    out=lps[32 * j:32 * j + 32, :],
                                    lhsT=hs[:, 32 * k:32 * k + 32],
                                    rhs=ws[:, g * FG:(g + 1) * FG],
                                    start=(k == 0 and ti == 0),
                                    stop=(k == PPOS - 1 and ti == 2),
                                    tile_position=(0, 32 * j),
                                )
                        else:
                            nc.tensor.matmul(
                                out=lps[32 * j:32 * j + 32, :],
                                lhsT=hpc[:, 32 * k:32 * k + 32],
                                rhs=wt_sb[:, g * FG:(g + 1) * FG],
                                start=(k == 0), stop=(k == PPOS - 1),
                                tile_position=(0, 32 * j),
                            )

                # ---- stage + output ----
                stg = stpool.tile([128, FG], FP, tag="stg")
                nc.scalar.copy(out=stg[:, :], in_=lps[:, :])
                dst = AP(out_dram,
                         (c * BCH) * (out_steps * VLOC) + (t % out_steps) * VLOC,
                         [[VG * PPOS, 4], [VG, PPOS],
                          [out_steps * VLOC, BCH], [1, VG]])
                nc.scalar.dma_start(out=dst, in_=stg[:, 0:VG])

                if t <= n_steps - 2:
                    # ---- local argmax ----
                    mx8 = wpool.tile([128, 8], FP, tag="mx8")
                    nc.vector.max(out=mx8[:, :], in_=lps[:, :])
                    mi8 = wpool.tile([128, 8], mybir.dt.uint32, tag="mi8")
                    nc.vector.max_index(out=mi8[:, :], in_max=mx8[:, :],
                                        in_values=lps[:, :])
                    cwi = wpool.tile([128, 1], FP, tag="cwi")
                    nc.vector.tensor_scalar_add(cwi[:, :], mi8[:, 0:1],
                                                off3_sb[:, 0:1])
                    tj = ptpool.tile([1, 256], FP, tag="tj")
                    tjv = tj[:, 0:128]
                    tji = tj[:, 128:256]
                    nc.tensor.transpose(out=tjv, in_=mx8[:, 0:1],
                                        identity=ident_sb[:, :])
